# revision 11
# baseline (speedup 1.0000x reference)
"""GatedDeltaNet on 8 Trainium2 NeuronCores (Bass/Tile).

Sharding: 2 batches x 4 cores. Each core owns 3 units of (head, dv-half) of
its batch: slot A = a full head (both dv halves), slot B = one half of a
shared head (pair-exchanged with the neighbor core for the RMS norm).

Per-core pipeline (one NEFF, all cores run the same program):
  A: projections from feature-major x^T (bf16 matmuls), causal dwconv+SiLU
     (feature-major, scalar_tensor_tensor taps), l2norm (ones-matmul
     partition reduction), beta/g rows, per-chunk g-cumsum (tensor scan).
  B: per chunk (C=128) per head-slot: decay matrices E via ACT exp of
     G_i - G_j, M = -(KK^T o E), Tmat^T = ((I+M)^{-1})^T via nilpotent
     Neumann doubling (bf16 matmuls), P^T, K~ (chunk-local), TKg^T, Qg^T.
  C: sequential chunk chain per unit: u = Tmat(v - Kg S), o = Qg S + P u,
     S = exp(G_C) S + K~^T u (fp32 master state, bf16 matmul shadow).
  C2: RMS norm (pair AllReduce for the straddling head's sum-of-squares),
     SiLU gate, PE-transpose o -> feature-major.
  D: output projection (norm_w folded into Wo) + grouped ReduceScatter.
"""
import numpy as np
import ml_dtypes

BF = ml_dtypes.bfloat16

# model dims
H, DK, DV, HID, CONV = 6, 256, 512, 2048, 4
NORM_EPS = 1e-5
L2_EPS = 1e-6

# kernel config
N_CORES = 8
T = 4096            # per batch
TB = 512            # phase-A time tile
NT = T // TB        # 8
KS = HID // 128     # 16 contraction slices
C = 128             # chunk length
NCH = T // C        # 32
DVU = 256           # dv per unit
NLVL = 4            # Neumann factors (I+Mn)(I+Mn^2)(I+Mn^4)(I+Mn^8)
SU = (0, 0, 1)      # unit -> head slot

# per-core unit tables: group g -> (headA, headB, dvh of unit2)
GROUPS = [
    (0, 1, 0),
    (2, 1, 1),
    (3, 4, 0),
    (5, 4, 1),
]

_BUILD_CACHE = {}


def _build():
    if "nc" in _BUILD_CACHE:
        return _BUILD_CACHE["nc"]
    import concourse.bass as bass  # noqa
    import concourse.bacc as bacc
    import concourse.mybir as mybir
    import concourse.tile as tile
    from contextlib import ExitStack

    F32 = mybir.dt.float32
    BF16 = mybir.dt.bfloat16
    AF = mybir.ActivationFunctionType
    ALU = mybir.AluOpType

    nc = bacc.Bacc("TRN2", target_bir_lowering=False, debug=False,
                   num_devices=N_CORES)

    xt = nc.dram_tensor("xt", [HID, T], BF16, kind="ExternalInput")
    w_qk = nc.dram_tensor("w_qk", [HID, 1024], BF16, kind="ExternalInput")
    w_ab = nc.dram_tensor("w_ab", [HID, 128], BF16, kind="ExternalInput")
    w_vg = nc.dram_tensor("w_vg", [HID, 1536], BF16, kind="ExternalInput")
    w_o = nc.dram_tensor("w_o", [768, HID], BF16, kind="ExternalInput")
    cst = nc.dram_tensor("cst", [128, 64], F32, kind="ExternalInput")
    msk = nc.dram_tensor("msk", [128, 512], F32, kind="ExternalInput")
    idf = nc.dram_tensor("idf", [128, 128], F32, kind="ExternalInput")
    idb = nc.dram_tensor("idb", [128, 128], BF16, kind="ExternalInput")
    out = nc.dram_tensor("out", [1024, HID], F32, kind="ExternalOutput")

    with tile.TileContext(nc) as tc, ExitStack() as ctx:
        pers = ctx.enter_context(tc.tile_pool(name="pers", bufs=1))
        dram = ctx.enter_context(tc.tile_pool(name="dram", bufs=1, space="DRAM"))

        # persistent SBUF
        brows = pers.tile([64, T], F32)
        Grows = pers.tile([64, T], F32)
        ssq = pers.tile([128, 3 * NCH], F32)
        ssq2x = pers.tile([128, NCH], F32)
        S_f = [pers.tile([128, 2, DVU], F32, name=f"S_f{u}") for u in range(3)]
        S_b = [pers.tile([128, 2, DVU], BF16, name=f"S_b{u}") for u in range(3)]
        cst_sb = pers.tile([128, 64], F32)
        msk_sb = pers.tile([128, 512], F32)
        idf_sb = pers.tile([128, 128], F32)
        idb_sb = pers.tile([128, 128], BF16)
        ones_f_row = pers.tile([33, 128], F32)
        ones_b_row = pers.tile([33, 128], BF16)
        ones_b_col = pers.tile([128, 1], BF16)

        nc.sync.dma_start(out=cst_sb, in_=cst[:])
        nc.sync.dma_start(out=msk_sb, in_=msk[:])
        nc.sync.dma_start(out=idf_sb, in_=idf[:])
        nc.sync.dma_start(out=idb_sb, in_=idb[:])
        nc.vector.memset(ones_f_row, 1.0)
        nc.vector.memset(ones_b_row, 1.0)
        nc.vector.memset(ones_b_col, 1.0)
        for u in range(3):
            nc.vector.memset(S_f[u], 0.0)
            nc.vector.memset(S_b[u], 0.0)

        # DRAM scratch
        qk_dr = dram.tile([1024, T], BF16)      # rows s*512 + qk*256 + f
        v_dr = dram.tile([768, T], BF16)        # rows u*256 + f
        gate_dr = dram.tile([T, 768], BF16)     # cols u*256 + dv
        o_dr = dram.tile([T, 768], BF16)
        ot_dr = dram.tile([768, T], BF16)
        ssq_cc_in = dram.tile([128, NCH], F32)
        ssq_cc_out = dram.tile([128, NCH], F32)
        po_st = [dram.tile([1024, HID], BF16, name=f"po_st{i}") for i in range(4)]
        rs_st = [dram.tile([256, HID], BF16, name=f"rs_st{i}") for i in range(4)]

        # conv halo carries (projection values of the last 3 steps)
        carries = {}
        for s in range(2):
            for qk in range(2):
                for ft in range(2):
                    carries[("qk", s, qk, ft)] = pers.tile([128, 3], BF16, name=f"cr_qk{s}{qk}{ft}")
        for u in range(3):
            for ft in range(2):
                carries[("v", u, ft)] = pers.tile([128, 3], BF16, name=f"cr_v{u}{ft}")
        for t_ in carries.values():
            nc.vector.memset(t_, 0.0)

        # ================= PHASE A =================
        with ExitStack() as actx:
            wp = actx.enter_context(tc.tile_pool(name="wp", bufs=1))
            xp = actx.enter_context(tc.tile_pool(name="xp", bufs=2))
            cp = actx.enter_context(tc.tile_pool(name="cp", bufs=2))
            pa = actx.enter_context(tc.tile_pool(name="pa", bufs=2, space="PSUM"))
            pb_ = actx.enter_context(tc.tile_pool(name="pb", bufs=1, space="PSUM"))

            w_qk_sb = wp.tile([128, KS, 1024], BF16)
            nc.sync.dma_start(out=w_qk_sb, in_=w_qk[:].rearrange("(k p) c -> p k c", p=128))
            w_ab_sb = wp.tile([128, KS, 128], BF16)
            nc.sync.dma_start(out=w_ab_sb, in_=w_ab[:].rearrange("(k p) c -> p k c", p=128))
            w_vg_sb = wp.tile([128, KS, 1536], BF16)
            nc.sync.dma_start(out=w_vg_sb, in_=w_vg[:].rearrange("(k p) c -> p k c", p=128))

            def conv_silu(ps, stream, wcol0):
                """ps: PSUM [128, TB] projection; returns SBUF bf16 [128, TB]."""
                pbuf = cp.tile([128, TB + 3], BF16, tag="pbuf")
                nc.vector.tensor_copy(out=pbuf[:, 0:3], in_=carries[stream])
                nc.vector.tensor_copy(out=pbuf[:, 3:TB + 3], in_=ps)
                nc.vector.tensor_copy(out=carries[stream], in_=pbuf[:, TB:TB + 3])
                cv = cp.tile([128, TB], BF16, tag="cv")
                nc.vector.tensor_scalar_mul(cv, pbuf[:, 0:TB], cst_sb[:, wcol0:wcol0 + 1])
                for j in range(1, 4):
                    nc.vector.scalar_tensor_tensor(
                        out=cv, in0=pbuf[:, j:j + TB],
                        scalar=cst_sb[:, wcol0 + j:wcol0 + j + 1], in1=cv,
                        op0=ALU.mult, op1=ALU.add)
                ee = cp.tile([128, TB], F32, tag="ee")
                nc.scalar.activation(out=ee, in_=cv, func=AF.Exp, scale=-1.0)
                nc.vector.tensor_scalar_add(ee, ee, 1.0)
                nc.vector.reciprocal(ee, ee)
                sl_ = cp.tile([128, TB], BF16, tag=f"sl{stream[-1]}")
                nc.vector.tensor_mul(sl_, cv, ee)
                return sl_

            for tt in range(NT):
                t0 = tt * TB
                x_t = xp.tile([128, KS, TB], BF16)
                nc.sync.dma_start(out=x_t, in_=xt[:, t0:t0 + TB].rearrange("(k p) t -> p k t", p=128))

                # ---- q/k (feature-major) + conv + silu + l2norm ----
                for s in range(2):
                    for qk in range(2):
                        sls = []
                        ps_ss = pb_.tile([1, TB], F32, tag="ss")
                        for ft in range(2):
                            ps = pa.tile([128, TB], F32, tag="proj")
                            col = s * 512 + qk * 256 + ft * 128
                            for k in range(KS):
                                nc.tensor.matmul(ps, w_qk_sb[:, k, col:col + 128],
                                                 x_t[:, k, :], start=(k == 0), stop=(k == KS - 1))
                            sl_ = conv_silu(ps, ("qk", s, qk, ft), s * 16 + qk * 8 + ft * 4)
                            sq = cp.tile([128, TB], BF16, tag="sq")
                            nc.vector.tensor_mul(sq, sl_, sl_)
                            nc.tensor.matmul(ps_ss, ones_b_col, sq,
                                             start=(ft == 0), stop=(ft == 1))
                            sls.append(sl_)
                        rs_row = cp.tile([1, TB], F32, tag="rsr")
                        if qk == 0:  # q: 1/sqrt(DK*(ss+eps))
                            nc.scalar.activation(out=rs_row, in_=ps_ss, func=AF.Ln,
                                                 scale=float(DK), bias=cst_sb[0:1, 59:60])
                        else:
                            nc.scalar.activation(out=rs_row, in_=ps_ss, func=AF.Ln,
                                                 scale=1.0, bias=cst_sb[0:1, 60:61])
                        nc.scalar.activation(out=rs_row, in_=rs_row, func=AF.Exp,
                                             scale=-0.5)
                        rs_bf = cp.tile([1, TB], BF16, tag="rsb")
                        nc.vector.tensor_copy(out=rs_bf, in_=rs_row)
                        ps_bc = pb_.tile([128, TB], F32, tag="bc")
                        nc.tensor.matmul(ps_bc, ones_b_row[0:1, :], rs_bf, start=True, stop=True)
                        for ft in range(2):
                            qn = cp.tile([128, TB], BF16, tag="qn")
                            nc.vector.tensor_mul(qn, sls[ft], ps_bc)
                            nc.sync.dma_start(
                                out=qk_dr[s * 512 + qk * 256 + ft * 128:
                                          s * 512 + qk * 256 + (ft + 1) * 128, t0:t0 + TB],
                                in_=qn)

                # ---- a/b rows -> beta, g, G, exp(G) ----
                ps_ab = pb_.tile([128, TB], F32, tag="ab")
                for k in range(KS):
                    nc.tensor.matmul(ps_ab, w_ab_sb[:, k, :], x_t[:, k, :],
                                     start=(k == 0), stop=(k == KS - 1))
                zr = cp.tile([1, TB], F32, tag="zr")
                nc.vector.memset(zr, 0.0)
                for s in range(2):
                    R = 32 * s
                    bt_ = cp.tile([1, TB], F32, tag="bt_")
                    nc.scalar.activation(out=bt_, in_=ps_ab[64 + R:65 + R, :],
                                         func=AF.Exp, scale=-1.0)
                    nc.vector.tensor_scalar_add(bt_, bt_, 1.0)
                    nc.vector.reciprocal(brows[R:R + 1, t0:t0 + TB], bt_)
                    gtmp = cp.tile([1, TB], F32, tag="gtmp")
                    nc.scalar.activation(out=gtmp, in_=ps_ab[R:R + 1, :],
                                         func=AF.Exp, bias=cst_sb[R:R + 1, 56:57])
                    nc.scalar.activation(out=gtmp, in_=gtmp, func=AF.Ln, bias=1.0)
                    nc.vector.tensor_scalar_mul(gtmp, gtmp, cst_sb[R:R + 1, 57:58])
                    for cc in range(TB // C):
                        a0 = t0 + cc * C
                        nc.vector.tensor_tensor_scan(
                            out=Grows[R:R + 1, a0:a0 + C],
                            data0=gtmp[:, cc * C:(cc + 1) * C],
                            data1=zr[:, 0:C], initial=0.0, op0=ALU.add, op1=ALU.add)

                # ---- v (feature-major) + conv + silu -> spill ----
                for u in range(3):
                    for ft in range(2):
                        ps = pa.tile([128, TB], F32, tag="proj")
                        col = u * 512 + ft * 128
                        for k in range(KS):
                            nc.tensor.matmul(ps, w_vg_sb[:, k, col:col + 128],
                                             x_t[:, k, :], start=(k == 0), stop=(k == KS - 1))
                        sl_ = conv_silu(ps, ("v", u, ft), 32 + u * 8 + ft * 4)
                        nc.sync.dma_start(
                            out=v_dr[u * 256 + ft * 128:u * 256 + (ft + 1) * 128,
                                     t0:t0 + TB],
                            in_=sl_)

                # ---- gate (time-major) + silu -> spill ----
                for st in range(TB // 128):
                    for u in range(3):
                        ps = pa.tile([128, 256], F32, tag="gate")
                        col = u * 512 + 256
                        for k in range(KS):
                            nc.tensor.matmul(ps, x_t[:, k, st * 128:(st + 1) * 128],
                                             w_vg_sb[:, k, col:col + 256],
                                             start=(k == 0), stop=(k == KS - 1))
                        ge = cp.tile([128, 256], F32, tag="ge")
                        nc.scalar.activation(out=ge, in_=ps, func=AF.Exp, scale=-1.0)
                        nc.vector.tensor_scalar_add(ge, ge, 1.0)
                        nc.vector.reciprocal(ge, ge)
                        gl = cp.tile([128, 256], BF16, tag="gl")
                        nc.vector.tensor_mul(gl, ps, ge)
                        nc.sync.dma_start(
                            out=gate_dr[t0 + st * 128:t0 + (st + 1) * 128,
                                        u * 256:(u + 1) * 256],
                            in_=gl)

        # ================= PHASE B + C =================
        with ExitStack() as bctx:
            bp = bctx.enter_context(tc.tile_pool(name="bp", bufs=4))
            ep = bctx.enter_context(tc.tile_pool(name="ep", bufs=4))
            vp = bctx.enter_context(tc.tile_pool(name="vp", bufs=3))
            pm = bctx.enter_context(tc.tile_pool(name="pm", bufs=3, space="PSUM"))
            pt = bctx.enter_context(tc.tile_pool(name="pt", bufs=2, space="PSUM"))
            pc = bctx.enter_context(tc.tile_pool(name="pc", bufs=3, space="PSUM"))

            for c in range(NCH):
                a0 = c * C
                sl = slice(a0, a0 + C)
                # shared per-chunk: transpose [G;beta] -> cols
                gb = ep.tile([97, C], F32, tag="gb")
                nc.vector.memset(gb, 0.0)
                nc.vector.tensor_copy(out=gb[0:1, :], in_=Grows[0:1, sl])
                nc.vector.tensor_copy(out=gb[32:33, :], in_=Grows[32:33, sl])
                nc.vector.tensor_copy(out=gb[64:65, :], in_=brows[0:1, sl])
                nc.vector.tensor_copy(out=gb[96:97, :], in_=brows[32:33, sl])
                ps_t = pm.tile([128, 97], F32, tag="pm")
                nc.tensor.transpose(ps_t, gb, idf_sb[0:97, 0:97])
                cols = ep.tile([128, 97], F32, tag="cols")
                nc.vector.tensor_copy(out=cols, in_=ps_t)

                slot = {}
                for s in range(2):
                    # qk slices from DRAM
                    qs = vp.tile([128, 2, C], BF16, tag="qs")
                    nc.sync.dma_start(out=qs, in_=qk_dr[s * 512:s * 512 + 256, sl]
                                      .rearrange("(f p) t -> p f t", p=128))
                    ks_ = vp.tile([128, 2, C], BF16, tag="ks")
                    nc.sync.dma_start(out=ks_, in_=qk_dr[s * 512 + 256:s * 512 + 512, sl]
                                      .rearrange("(f p) t -> p f t", p=128))

                    gcol = cols[:, 32 * s:32 * s + 1]
                    bcol = cols[:, 64 + 32 * s:65 + 32 * s]
                    # G_C broadcast col + derived scalars
                    ps_gc = pm.tile([128, 1], F32, tag="pm")
                    R = 32 * s
                    nc.tensor.matmul(ps_gc, ones_f_row[R:R + 1, :],
                                     Grows[R:R + 1, a0 + C - 1:a0 + C],
                                     start=True, stop=True)
                    gcc = ep.tile([128, 1], F32, tag="gcc")
                    nc.vector.tensor_copy(out=gcc, in_=ps_gc)
                    eGC = ep.tile([128, 1], F32, tag="eGC")
                    nc.scalar.activation(out=eGC, in_=gcc, func=AF.Exp)
                    ktsc = ep.tile([128, 1], F32, tag="ktsc")
                    nc.scalar.activation(out=ktsc, in_=gcol, func=AF.Exp,
                                         scale=-1.0, bias=gcc)
                    nc.vector.tensor_mul(ktsc, ktsc, bcol)
                    egcol = ep.tile([128, 1], F32, tag="egcol")
                    nc.scalar.activation(out=egcol, in_=gcol, func=AF.Exp)

                    # replicated rows
                    ps_gr = pm.tile([128, C], F32, tag="pm")
                    nc.tensor.matmul(ps_gr, ones_f_row[R:R + 1, :], Grows[R:R + 1, sl],
                                     start=True, stop=True)
                    ps_br = pm.tile([128, C], F32, tag="pm")
                    nc.tensor.matmul(ps_br, ones_f_row[R:R + 1, :], brows[R:R + 1, sl],
                                     start=True, stop=True)

                    # E matrices
                    dd = ep.tile([128, C], F32, tag="dd")
                    nc.vector.scalar_tensor_tensor(out=dd, in0=ps_gr, scalar=gcol,
                                                   in1=msk_sb[:, 0:128],
                                                   op0=ALU.subtract, op1=ALU.add)
                    einc = ep.tile([128, C], F32, tag="einc")
                    nc.scalar.activation(out=einc, in_=dd, func=AF.Exp, scale=-1.0)
                    dd2 = ep.tile([128, C], F32, tag="dd2")
                    nc.vector.scalar_tensor_tensor(out=dd2, in0=ps_gr, scalar=gcol,
                                                   in1=msk_sb[:, 128:256],
                                                   op0=ALU.subtract, op1=ALU.add)
                    eups = ep.tile([128, C], F32, tag="eups")
                    nc.scalar.activation(out=eups, in_=dd2, func=AF.Exp)
                    dd3 = ep.tile([128, C], F32, tag="dd3")
                    nc.vector.scalar_tensor_tensor(out=dd3, in0=ps_gr, scalar=gcol,
                                                   in1=msk_sb[:, 256:384],
                                                   op0=ALU.subtract, op1=ALU.add)
                    eupi = ep.tile([128, C], F32, tag="eupi")
                    nc.scalar.activation(out=eupi, in_=dd3, func=AF.Exp)

                    ebinc = ep.tile([128, C], F32, tag="ebinc")
                    nc.vector.tensor_mul(ebinc, einc, ps_br)
                    ebsn = ep.tile([128, C], F32, tag="ebsn")
                    nc.vector.tensor_mul(ebsn, ebinc, msk_sb[:, 384:512])
                    nc.vector.tensor_scalar(out=eups, in0=eups, scalar1=bcol,
                                            scalar2=-1.0, op0=ALU.mult, op1=ALU.mult)
                    nc.vector.tensor_scalar_mul(eupi, eupi, bcol)

                    # KK / KQ
                    ps_kk = pm.tile([128, C], F32, tag="pm")
                    for ft in range(2):
                        nc.tensor.matmul(ps_kk, ks_[:, ft, :], ks_[:, ft, :],
                                         start=(ft == 0), stop=(ft == 1))
                    ps_kq = pm.tile([128, C], F32, tag="pm")
                    for ft in range(2):
                        nc.tensor.matmul(ps_kq, ks_[:, ft, :], qs[:, ft, :],
                                         start=(ft == 0), stop=(ft == 1))
                    Mn = bp.tile([128, C], BF16, tag="Mn")
                    nc.vector.tensor_mul(Mn, ebsn, ps_kk)
                    MnT = bp.tile([128, C], BF16, tag="MnT")
                    nc.vector.tensor_mul(MnT, eups, ps_kk)
                    PT = bp.tile([128, C], BF16, tag="PT")
                    nc.vector.tensor_mul(PT, eupi, ps_kq)

                    # T-chain: A = Tmat^T
                    A = bp.tile([128, C], BF16, tag="A0")
                    nc.vector.tensor_add(A, idb_sb, MnT)
                    ps_x = pm.tile([128, C], F32, tag="pm")
                    nc.tensor.matmul(ps_x, MnT, Mn, start=True, stop=True)
                    X = bp.tile([128, C], BF16, tag="X")
                    nc.vector.tensor_copy(out=X, in_=ps_x)
                    ps_x2 = pm.tile([128, C], F32, tag="pm")
                    nc.tensor.matmul(ps_x2, Mn, MnT, start=True, stop=True)
                    XT = bp.tile([128, C], BF16, tag="XT")
                    nc.vector.tensor_copy(out=XT, in_=ps_x2)
                    for lvl in range(NLVL - 1):
                        ps_a = pm.tile([128, C], F32, tag="pm")
                        nc.tensor.matmul(ps_a, X, A, start=True, stop=True)
                        A2 = bp.tile([128, C], BF16, tag=f"A{lvl + 1}")
                        nc.vector.tensor_add(A2, ps_a, A)
                        A = A2
                        if lvl < NLVL - 2:
                            ps_y = pm.tile([128, C], F32, tag="pm")
                            nc.tensor.matmul(ps_y, XT, X, start=True, stop=True)
                            ps_y2 = pm.tile([128, C], F32, tag="pm")
                            nc.tensor.matmul(ps_y2, X, XT, start=True, stop=True)
                            X = bp.tile([128, C], BF16, tag="X")
                            nc.vector.tensor_copy(out=X, in_=ps_y)
                            XT = bp.tile([128, C], BF16, tag="XT")
                            nc.vector.tensor_copy(out=XT, in_=ps_y2)

                    # K time-major + scaled variants
                    ktm = bp.tile([128, 256], BF16, tag="ktm")
                    for ft in range(2):
                        ps_tr = pt.tile([128, 128], BF16, tag="tr")
                        nc.tensor.transpose(ps_tr, ks_[:, ft, :], idb_sb)
                        nc.vector.tensor_copy(out=ktm[:, ft * 128:(ft + 1) * 128], in_=ps_tr)
                    kgn = bp.tile([128, 256], BF16, tag="kgn")
                    nc.vector.tensor_scalar(out=kgn, in0=ktm, scalar1=egcol,
                                            scalar2=-1.0, op0=ALU.mult, op1=ALU.mult)
                    ktl = bp.tile([128, 256], BF16, tag="ktl")
                    nc.vector.tensor_scalar_mul(ktl, ktm, ktsc)
                    tkgn = bp.tile([128, 256], BF16, tag="tkgn")
                    for ft in range(2):
                        ps_k = pm.tile([128, C], F32, tag="pm")
                        nc.tensor.matmul(ps_k, kgn[:, ft * 128:(ft + 1) * 128], A,
                                         start=True, stop=True)
                        nc.vector.tensor_copy(out=tkgn[:, ft * 128:(ft + 1) * 128], in_=ps_k)
                    egr = ep.tile([1, C], BF16, tag="egr")
                    nc.scalar.activation(out=egr, in_=Grows[R:R + 1, sl], func=AF.Exp)
                    ps_eg = pm.tile([128, C], F32, tag="pm")
                    nc.tensor.matmul(ps_eg, ones_b_row[0:1, :], egr,
                                     start=True, stop=True)
                    qgt = bp.tile([128, 256], BF16, tag="qgt")
                    for ft in range(2):
                        nc.vector.tensor_mul(qgt[:, ft * 128:(ft + 1) * 128],
                                             qs[:, ft, :], ps_eg)
                    slot[s] = dict(A=A, PT=PT, tkgn=tkgn, qgt=qgt, ktl=ktl, eGC=eGC)

                # ---- per-unit chain ----
                for u in range(3):
                    s = SU[u]
                    sd = slot[s]
                    vfa = vp.tile([128, 2, C], BF16, tag="vfa")
                    nc.sync.dma_start(out=vfa, in_=v_dr[u * 256:(u + 1) * 256, sl]
                                      .rearrange("(f p) t -> p f t", p=128))
                    vtm = vp.tile([128, 256], BF16, tag="vtm")
                    for ft in range(2):
                        ps_tr = pt.tile([128, 128], BF16, tag="tr")
                        nc.tensor.transpose(ps_tr, vfa[:, ft, :], idb_sb)
                        nc.vector.tensor_copy(out=vtm[:, ft * 128:(ft + 1) * 128], in_=ps_tr)

                    ps_u = pc.tile([128, DVU], F32, tag="pc")
                    nc.tensor.matmul(ps_u, sd["A"], vtm, start=True, stop=(c == 0))
                    if c > 0:
                        for ft in range(2):
                            nc.tensor.matmul(ps_u, sd["tkgn"][:, ft * 128:(ft + 1) * 128],
                                             S_b[u][:, ft, :], start=False, stop=(ft == 1))
                    u_sb = vp.tile([128, DVU], BF16, tag="usb")
                    nc.vector.tensor_copy(out=u_sb, in_=ps_u)

                    ps_o = pc.tile([128, DVU], F32, tag="pc")
                    if c > 0:
                        for ft in range(2):
                            nc.tensor.matmul(ps_o, sd["qgt"][:, ft * 128:(ft + 1) * 128],
                                             S_b[u][:, ft, :], start=(ft == 0), stop=False)
                    nc.tensor.matmul(ps_o, sd["PT"], u_sb, start=(c == 0), stop=True)
                    obf = vp.tile([128, DVU], BF16, tag="obf")
                    nc.vector.tensor_copy(out=obf, in_=ps_o)
                    nc.sync.dma_start(out=o_dr[sl, u * 256:(u + 1) * 256], in_=obf)
                    trash = vp.tile([128, DVU], BF16, tag="trash")
                    nc.scalar.activation(out=trash, in_=ps_o, func=AF.Square,
                                         accum_out=ssq[:, u * NCH + c:u * NCH + c + 1])

                    for ft in range(2):
                        ps_s = pc.tile([128, DVU], F32, tag="pc")
                        nc.tensor.matmul(ps_s, sd["ktl"][:, ft * 128:(ft + 1) * 128],
                                         u_sb, start=True, stop=True)
                        nc.vector.scalar_tensor_tensor(
                            out=S_f[u][:, ft, :], in0=S_f[u][:, ft, :],
                            scalar=sd["eGC"], in1=ps_s, op0=ALU.mult, op1=ALU.add)
                    nc.vector.tensor_copy(out=S_b[u], in_=S_f[u])

        # ---- ssq pair exchange ----
        nc.sync.dma_start(out=ssq_cc_in[:], in_=ssq[:, 2 * NCH:3 * NCH])
        nc.gpsimd.collective_compute(
            "AllReduce", mybir.AluOpType.add,
            replica_groups=[[0, 1], [2, 3], [4, 5], [6, 7]],
            ins=[ssq_cc_in[:]], outs=[ssq_cc_out[:]])
        nc.sync.dma_start(out=ssq2x, in_=ssq_cc_out[:])

        # ================= PHASE C2 =================
        with ExitStack() as cctx:
            np_ = cctx.enter_context(tc.tile_pool(name="np", bufs=3))
            pn = cctx.enter_context(tc.tile_pool(name="pn", bufs=2, space="PSUM"))
            for c in range(NCH):
                a0 = c * C
                sl = slice(a0, a0 + C)
                totA = np_.tile([128, 1], F32, tag="totA")
                nc.vector.tensor_add(totA, ssq[:, c:c + 1], ssq[:, NCH + c:NCH + c + 1])
                rsqA = np_.tile([128, 1], F32, tag="rsqA")
                nc.scalar.activation(out=rsqA, in_=totA, func=AF.Ln,
                                     scale=float(1.0 / DV), bias=cst_sb[:, 61:62])
                nc.scalar.activation(out=rsqA, in_=rsqA, func=AF.Exp, scale=-0.5)
                rsqB = np_.tile([128, 1], F32, tag="rsqB")
                nc.scalar.activation(out=rsqB, in_=ssq2x[:, c:c + 1], func=AF.Ln,
                                     scale=float(1.0 / DV), bias=cst_sb[:, 61:62])
                nc.scalar.activation(out=rsqB, in_=rsqB, func=AF.Exp, scale=-0.5)
                for u in range(3):
                    rsq = rsqA if u < 2 else rsqB
                    ot = np_.tile([128, DVU], BF16, tag="ot")
                    nc.sync.dma_start(out=ot, in_=o_dr[sl, u * 256:(u + 1) * 256])
                    gt = np_.tile([128, DVU], BF16, tag="gt")
                    nc.sync.dma_start(out=gt, in_=gate_dr[sl, u * 256:(u + 1) * 256])
                    otn = np_.tile([128, DVU], BF16, tag="otn")
                    nc.vector.scalar_tensor_tensor(out=otn, in0=ot, scalar=rsq,
                                                   in1=gt, op0=ALU.mult, op1=ALU.mult)
                    for ft in range(2):
                        ps_tr = pn.tile([128, 128], BF16, tag="tr")
                        nc.tensor.transpose(ps_tr, otn[:, ft * 128:(ft + 1) * 128], idb_sb)
                        otr = np_.tile([128, 128], BF16, tag="otr")
                        nc.vector.tensor_copy(out=otr, in_=ps_tr)
                        nc.sync.dma_start(
                            out=ot_dr[u * 256 + ft * 128:u * 256 + (ft + 1) * 128, sl],
                            in_=otr)

        # ================= PHASE D =================
        with ExitStack() as dctx:
            dp = dctx.enter_context(tc.tile_pool(name="dp", bufs=3))
            wop = dctx.enter_context(tc.tile_pool(name="wop", bufs=1))
            pd = dctx.enter_context(tc.tile_pool(name="pd", bufs=3, space="PSUM"))
            w_o_sb = wop.tile([128, 6, HID], BF16)
            nc.sync.dma_start(out=w_o_sb, in_=w_o[:].rearrange("(r p) n -> p r n", p=128))
            for st4 in range(4):
                for ti in range(8):
                    rows = st4 * 1024 + ti * 128
                    otl = dp.tile([128, 6, 128], BF16, tag="otl")
                    nc.sync.dma_start(out=otl, in_=ot_dr[:, rows:rows + 128]
                                      .rearrange("(r p) t -> p r t", p=128))
                    for nt_ in range(4):
                        ps = pd.tile([128, 512], F32, tag="pd")
                        for r in range(6):
                            nc.tensor.matmul(ps, otl[:, r, :],
                                             w_o_sb[:, r, nt_ * 512:(nt_ + 1) * 512],
                                             start=(r == 0), stop=(r == 5))
                        pob = dp.tile([128, 512], BF16, tag="pob")
                        nc.vector.tensor_copy(out=pob, in_=ps)
                        nc.sync.dma_start(out=po_st[st4][ti * 128:(ti + 1) * 128,
                                                         nt_ * 512:(nt_ + 1) * 512],
                                          in_=pob)
                nc.gpsimd.collective_compute(
                    "ReduceScatter", mybir.AluOpType.add,
                    replica_groups=[[0, 1, 2, 3], [4, 5, 6, 7]],
                    ins=[po_st[st4][:]], outs=[rs_st[st4][:]])
                for r2 in range(2):
                    rsb = dp.tile([128, HID], BF16, tag="rsb")
                    nc.sync.dma_start(out=rsb, in_=rs_st[st4][r2 * 128:(r2 + 1) * 128, :])
                    rsf = dp.tile([128, HID], F32, tag="rsf")
                    nc.vector.tensor_copy(out=rsf, in_=rsb)
                    nc.sync.dma_start(out=out[st4 * 256 + r2 * 128:
                                              st4 * 256 + (r2 + 1) * 128, :], in_=rsf)

    nc.compile()
    _BUILD_CACHE["nc"] = nc
    return nc


def _prep_core_inputs(ins, core):
    """Pack per-core input arrays. ins: dict of fp32 numpy arrays."""
    b, g = core // 4, core % 4
    hA, hB, dvhB = GROUPS[g]
    units = [(hA, 0), (hA, 1), (hB, dvhB)]
    heads = [hA, hB]

    x = np.asarray(ins["hidden_states"], np.float32)[b]        # [T, HID]
    xt = np.ascontiguousarray(x.T).astype(BF)                  # [HID, T]

    w_qk = np.zeros((HID, 1024), np.float32)
    w_ab = np.zeros((HID, 128), np.float32)
    for s, h in enumerate(heads):
        w_qk[:, s * 512:s * 512 + 256] = ins["Wq"][h * DK:(h + 1) * DK].T
        w_qk[:, s * 512 + 256:s * 512 + 512] = ins["Wk"][h * DK:(h + 1) * DK].T
        w_ab[:, 32 * s] = ins["Wa"][h]
        w_ab[:, 64 + 32 * s] = ins["Wb"][h]

    w_vg = np.zeros((HID, 1536), np.float32)
    w_o = np.zeros((768, HID), np.float32)
    for u, (h, dvh) in enumerate(units):
        r = slice(h * DV + dvh * 256, h * DV + dvh * 256 + 256)
        w_vg[:, u * 512:u * 512 + 256] = ins["Wv"][r].T
        w_vg[:, u * 512 + 256:u * 512 + 512] = ins["Wg"][r].T
        nw = ins["norm_w"][dvh * 256:(dvh + 1) * 256]
        w_o[u * 256:(u + 1) * 256, :] = (ins["Wo"][:, r].T * nw[:, None])

    cst = np.zeros((128, 64), np.float32)
    for s, h in enumerate(heads):
        for qk, cw in ((0, ins["conv_wq"]), (1, ins["conv_wk"])):
            for ft in range(2):
                for j in range(4):
                    cst[:, s * 16 + qk * 8 + ft * 4 + j] = \
                        cw[h * DK + ft * 128:h * DK + (ft + 1) * 128, j]
    for u, (h, dvh) in enumerate(units):
        for ft in range(2):
            for j in range(4):
                cst[:, 32 + u * 8 + ft * 4 + j] = \
                    ins["conv_wv"][h * DV + dvh * 256 + ft * 128:
                                   h * DV + dvh * 256 + (ft + 1) * 128, j]
    for s, h in enumerate(heads):
        cst[32 * s, 56] = ins["dt_bias"][h]
        cst[32 * s, 57] = -np.exp(ins["A_log"][h])
    cst[:, 58] = 1.0
    cst[:, 59] = DK * L2_EPS
    cst[:, 60] = L2_EPS
    cst[:, 61] = NORM_EPS

    ii, jj = np.mgrid[0:128, 0:128]
    msk = np.zeros((128, 512), np.float32)
    msk[:, 0:128] = np.where(jj > ii, 1e9, 0.0)        # pre-exp(-x) incl-lower
    msk[:, 128:256] = np.where(jj <= ii, -1e9, 0.0)    # pre-exp(+x) strict-upper
    msk[:, 256:384] = np.where(jj < ii, -1e9, 0.0)     # pre-exp(+x) incl-upper
    msk[:, 384:512] = np.where(jj < ii, -1.0, 0.0)     # -(strict lower 0/1)

    idf = np.eye(128, dtype=np.float32)

    return {
        "xt": np.ascontiguousarray(xt),
        "w_qk": np.ascontiguousarray(w_qk.astype(BF)),
        "w_ab": np.ascontiguousarray(w_ab.astype(BF)),
        "w_vg": np.ascontiguousarray(w_vg.astype(BF)),
        "w_o": np.ascontiguousarray(w_o.astype(BF)),
        "cst": cst,
        "msk": msk,
        "idf": idf,
        "idb": np.ascontiguousarray(idf.astype(BF)),
    }


LAST_RESULTS = None


def _run_device(ins, trace=False):
    global LAST_RESULTS
    from concourse.bass_utils import run_bass_kernel_spmd
    nc = _build()
    in_maps = [_prep_core_inputs(ins, c) for c in range(N_CORES)]
    res = run_bass_kernel_spmd(nc, in_maps, list(range(N_CORES)), trace=trace)
    LAST_RESULTS = res
    B = 2
    full = np.empty((B, T, HID), np.float32)
    for core in range(N_CORES):
        b, g = core // 4, core % 4
        o = res.results[core]["out"]                # [1024, HID]
        for st4 in range(4):
            full[b, st4 * 1024 + g * 256: st4 * 1024 + (g + 1) * 256] = \
                o[st4 * 256:(st4 + 1) * 256]
    return full


def kernel(**inputs):
    ins = {k: np.asarray(v, np.float32) for k, v in inputs.items()}
    return _run_device(ins)


# revision 14
# speedup vs baseline: 1.1625x; 1.1625x over previous
"""GatedDeltaNet on 8 Trainium2 NeuronCores (Bass/Tile).

Sharding: 2 batches x 4 cores. Each core owns 3 units of (head, dv-half) of
its batch: slot A = a full head (both dv halves), slot B = one half of a
shared head (pair-exchanged with the neighbor core for the RMS norm).

Per-core pipeline (one NEFF, all cores run the same program):
  A: projections from feature-major x^T (bf16 matmuls), causal dwconv+SiLU
     (feature-major, scalar_tensor_tensor taps), l2norm (ones-matmul
     partition reduction), beta/g rows, per-chunk g-cumsum (tensor scan).
  B: per chunk (C=128) per head-slot: decay matrices E via ACT exp of
     G_i - G_j, M = -(KK^T o E), Tmat^T = ((I+M)^{-1})^T via nilpotent
     Neumann doubling (bf16 matmuls), P^T, K~ (chunk-local), TKg^T, Qg^T.
  C: sequential chunk chain per unit: u = Tmat(v - Kg S), o = Qg S + P u,
     S = exp(G_C) S + K~^T u (fp32 master state, bf16 matmul shadow).
  C2: RMS norm (pair AllReduce for the straddling head's sum-of-squares),
     SiLU gate, PE-transpose o -> feature-major.
  D: output projection (norm_w folded into Wo) + grouped ReduceScatter.
"""
import numpy as np
import ml_dtypes

BF = ml_dtypes.bfloat16

# model dims
H, DK, DV, HID, CONV = 6, 256, 512, 2048, 4
NORM_EPS = 1e-5
L2_EPS = 1e-6

# kernel config
N_CORES = 8
T = 4096            # per batch
TB = 512            # phase-A time tile
NT = T // TB        # 8
KS = HID // 128     # 16 contraction slices
C = 128             # chunk length
NCH = T // C        # 32
DVU = 256           # dv per unit
NLVL = 3            # Neumann factors (I+Mn)(I+Mn^2)(I+Mn^4)
SU = (0, 0, 1)      # unit -> head slot

# per-core unit tables: group g -> (headA, headB, dvh of unit2)
GROUPS = [
    (0, 1, 0),
    (2, 1, 1),
    (3, 4, 0),
    (5, 4, 1),
]

_BUILD_CACHE = {}


def _build():
    if "nc" in _BUILD_CACHE:
        return _BUILD_CACHE["nc"]
    import concourse.bass as bass  # noqa
    import concourse.bacc as bacc
    import concourse.mybir as mybir
    import concourse.tile as tile
    from contextlib import ExitStack

    F32 = mybir.dt.float32
    BF16 = mybir.dt.bfloat16
    AF = mybir.ActivationFunctionType
    ALU = mybir.AluOpType

    nc = bacc.Bacc("TRN2", target_bir_lowering=False, debug=False,
                   num_devices=N_CORES)

    xt = nc.dram_tensor("xt", [HID, T], BF16, kind="ExternalInput")
    w_qk = nc.dram_tensor("w_qk", [HID, 1024], BF16, kind="ExternalInput")
    w_ab = nc.dram_tensor("w_ab", [HID, 128], BF16, kind="ExternalInput")
    w_vg = nc.dram_tensor("w_vg", [HID, 1536], BF16, kind="ExternalInput")
    w_o = nc.dram_tensor("w_o", [768, HID], BF16, kind="ExternalInput")
    cst = nc.dram_tensor("cst", [128, 64], F32, kind="ExternalInput")
    msk = nc.dram_tensor("msk", [128, 384], F32, kind="ExternalInput")
    idf = nc.dram_tensor("idf", [128, 128], F32, kind="ExternalInput")
    idb = nc.dram_tensor("idb", [128, 128], BF16, kind="ExternalInput")
    out = nc.dram_tensor("out", [1024, HID], F32, kind="ExternalOutput")

    with tile.TileContext(nc) as tc, ExitStack() as ctx:
        pers = ctx.enter_context(tc.tile_pool(name="pers", bufs=1))
        dram = ctx.enter_context(tc.tile_pool(name="dram", bufs=1, space="DRAM"))

        # persistent SBUF
        brows = pers.tile([64, T], F32)
        brows_neg = pers.tile([64, T], F32)
        Grows = pers.tile([64, T], F32)
        ssq = pers.tile([128, 3 * NCH], F32)
        ssq2x = pers.tile([128, NCH], F32)
        S_f = [pers.tile([128, 2, DVU], F32, name=f"S_f{u}") for u in range(3)]
        S_b = [pers.tile([128, 2, DVU], BF16, name=f"S_b{u}") for u in range(3)]
        cst_sb = pers.tile([128, 64], F32)
        msk_sb = pers.tile([128, 384], F32)
        idf_sb = pers.tile([128, 128], F32)
        idb_sb = pers.tile([128, 128], BF16)
        ones_f_row = pers.tile([33, 128], F32)
        ones_b_row = pers.tile([33, 128], BF16)
        ones_b_col = pers.tile([128, 1], BF16)

        nc.sync.dma_start(out=cst_sb, in_=cst[:])
        nc.sync.dma_start(out=msk_sb, in_=msk[:])
        nc.sync.dma_start(out=idf_sb, in_=idf[:])
        nc.sync.dma_start(out=idb_sb, in_=idb[:])
        nc.vector.memset(ones_f_row, 1.0)
        nc.vector.memset(ones_b_row, 1.0)
        nc.vector.memset(ones_b_col, 1.0)
        for u in range(3):
            nc.vector.memset(S_f[u], 0.0)
            nc.vector.memset(S_b[u], 0.0)

        # DRAM scratch
        qk_dr = dram.tile([1024, T], BF16)      # rows s*512 + qk*256 + f
        v_dr = dram.tile([768, T], BF16)        # rows u*256 + f
        gate_dr = dram.tile([T, 768], BF16)     # cols u*256 + dv
        o_dr = dram.tile([T, 768], BF16)
        ot_dr = dram.tile([768, T], BF16)
        ssq_cc_in = dram.tile([128, NCH], F32)
        ssq_cc_out = dram.tile([128, NCH], F32)
        po_st = [dram.tile([1024, HID], BF16, name=f"po_st{i}") for i in range(4)]
        rs_st = [dram.tile([256, HID], BF16, name=f"rs_st{i}") for i in range(4)]

        # conv halo carries (projection values of the last 3 steps)
        carries = {}
        for s in range(2):
            for qk in range(2):
                for ft in range(2):
                    carries[("qk", s, qk, ft)] = pers.tile([128, 3], BF16, name=f"cr_qk{s}{qk}{ft}")
        for u in range(3):
            for ft in range(2):
                carries[("v", u, ft)] = pers.tile([128, 3], BF16, name=f"cr_v{u}{ft}")
        for t_ in carries.values():
            nc.vector.memset(t_, 0.0)

        # ================= PHASE A =================
        def make_conv_silu(cp):
            def conv_silu(ps, stream, wcol0, out=None):
                """ps: PSUM [128, TB] projection -> causal conv + SiLU, bf16."""
                pbuf = cp.tile([128, TB + 3], BF16, tag="pbuf", name="pbuf")
                nc.gpsimd.tensor_copy(out=pbuf[:, 0:3], in_=carries[stream])
                nc.scalar.copy(out=pbuf[:, 3:TB + 3], in_=ps)
                nc.gpsimd.tensor_copy(out=carries[stream], in_=pbuf[:, TB:TB + 3])
                cv = cp.tile([128, TB], BF16, tag="cv", name="cv")
                nc.vector.tensor_scalar_mul(cv, pbuf[:, 0:TB], cst_sb[:, wcol0:wcol0 + 1])
                for j in range(1, 4):
                    nc.vector.scalar_tensor_tensor(
                        out=cv, in0=pbuf[:, j:j + TB],
                        scalar=cst_sb[:, wcol0 + j:wcol0 + j + 1], in1=cv,
                        op0=ALU.mult, op1=ALU.add)
                ee = cp.tile([128, TB], BF16, tag="ee", name="ee")
                nc.scalar.activation(out=ee, in_=cv, func=AF.Exp, scale=-1.0)
                nc.gpsimd.tensor_scalar_add(ee, ee, 1.0)
                with nc.allow_low_precision("sigmoid denom bf16 ok at rms 2e-2"):
                    nc.vector.reciprocal(ee, ee)
                sl_ = out
                if sl_ is None:
                    sl_ = cp.tile([128, TB], BF16, tag=f"sl{stream[-1]}",
                                  name=f"sl{stream[-1]}")
                nc.vector.tensor_mul(sl_, cv, ee)
                return sl_
            return conv_silu

        # ---- pass A1: q/k + a/b ----
        with ExitStack() as actx:
            wp = actx.enter_context(tc.tile_pool(name="wp", bufs=1))
            xp = actx.enter_context(tc.tile_pool(name="xp", bufs=2))
            cp = actx.enter_context(tc.tile_pool(name="cp", bufs=2))
            pa = actx.enter_context(tc.tile_pool(name="pa", bufs=2, space="PSUM"))
            pb_ = actx.enter_context(tc.tile_pool(name="pb", bufs=1, space="PSUM"))
            conv_silu = make_conv_silu(cp)

            w_qk_sb = wp.tile([128, KS, 1024], BF16)
            nc.sync.dma_start(out=w_qk_sb, in_=w_qk[:].rearrange("(k p) c -> p k c", p=128))
            w_ab_sb = wp.tile([128, KS, 128], BF16)
            nc.sync.dma_start(out=w_ab_sb, in_=w_ab[:].rearrange("(k p) c -> p k c", p=128))

            for tt in range(NT):
                t0 = tt * TB
                x_t = xp.tile([128, KS, TB], BF16, name="x_t")
                nc.sync.dma_start(out=x_t, in_=xt[:, t0:t0 + TB].rearrange("(k p) t -> p k t", p=128))

                for s in range(2):
                    qn4 = cp.tile([128, 4, TB], BF16, tag="qn4", name="qn4")
                    for qk in range(2):
                        sls = []
                        ps_ss = pb_.tile([1, TB], F32, tag="ss", name="ps_ss")
                        for ft in range(2):
                            ps = pa.tile([128, TB], F32, tag="proj", name="ps_qk")
                            col = s * 512 + qk * 256 + ft * 128
                            for k in range(KS):
                                nc.tensor.matmul(ps, w_qk_sb[:, k, col:col + 128],
                                                 x_t[:, k, :], start=(k == 0), stop=(k == KS - 1))
                            sl_ = conv_silu(ps, ("qk", s, qk, ft), s * 16 + qk * 8 + ft * 4)
                            sq = cp.tile([128, TB], BF16, tag="sq", name="sq")
                            nc.vector.tensor_mul(sq, sl_, sl_)
                            nc.tensor.matmul(ps_ss, ones_b_col, sq,
                                             start=(ft == 0), stop=(ft == 1))
                            sls.append(sl_)
                        rs_row = cp.tile([1, TB], F32, tag="rsr", name="rs_row")
                        if qk == 0:  # q: 1/sqrt(DK*(ss+eps))
                            nc.scalar.activation(out=rs_row, in_=ps_ss, func=AF.Ln,
                                                 scale=float(DK), bias=cst_sb[0:1, 59:60])
                        else:
                            nc.scalar.activation(out=rs_row, in_=ps_ss, func=AF.Ln,
                                                 scale=1.0, bias=cst_sb[0:1, 60:61])
                        nc.scalar.activation(out=rs_row, in_=rs_row, func=AF.Exp,
                                             scale=-0.5)
                        rs_bf = cp.tile([1, TB], BF16, tag="rsb", name="rs_bf")
                        nc.vector.tensor_copy(out=rs_bf, in_=rs_row)
                        ps_bc = pb_.tile([128, TB], F32, tag="bc", name="ps_bc")
                        nc.tensor.matmul(ps_bc, ones_b_row[0:1, :], rs_bf, start=True, stop=True)
                        for ft in range(2):
                            nc.vector.tensor_mul(qn4[:, qk * 2 + ft, :], sls[ft], ps_bc)
                    nc.sync.dma_start(
                        out=qk_dr[s * 512:(s + 1) * 512, t0:t0 + TB]
                        .rearrange("(j p) t -> p j t", p=128),
                        in_=qn4)

                # ---- a/b rows -> beta, g, G ----
                ps_ab = pb_.tile([128, TB], F32, tag="ab", name="ps_ab")
                for k in range(KS):
                    nc.tensor.matmul(ps_ab, w_ab_sb[:, k, :], x_t[:, k, :],
                                     start=(k == 0), stop=(k == KS - 1))
                zr = cp.tile([1, 128], F32, tag="zr", name="zr")
                nc.vector.memset(zr, 0.0)
                for s in range(2):
                    R = 32 * s
                    bt_ = cp.tile([1, TB], F32, tag="abtmp", name="bt_")
                    nc.scalar.activation(out=bt_, in_=ps_ab[64 + R:65 + R, :],
                                         func=AF.Exp, scale=-1.0)
                    nc.vector.tensor_scalar_add(bt_, bt_, 1.0)
                    nc.vector.reciprocal(brows[R:R + 1, t0:t0 + TB], bt_)
                    nc.vector.tensor_scalar_mul(brows_neg[R:R + 1, t0:t0 + TB],
                                                brows[R:R + 1, t0:t0 + TB], -1.0)
                    gtmp = cp.tile([1, TB], F32, tag="abtmp", name="gtmp")
                    nc.scalar.activation(out=gtmp, in_=ps_ab[R:R + 1, :],
                                         func=AF.Exp, bias=cst_sb[R:R + 1, 56:57])
                    nc.scalar.activation(out=gtmp, in_=gtmp, func=AF.Ln, bias=1.0)
                    nc.vector.tensor_scalar_mul(gtmp, gtmp, cst_sb[R:R + 1, 57:58])
                    for cc in range(TB // C):
                        a0 = t0 + cc * C
                        nc.vector.tensor_tensor_scan(
                            out=Grows[R:R + 1, a0:a0 + C],
                            data0=gtmp[:, cc * C:(cc + 1) * C],
                            data1=zr[:, 0:C], initial=0.0, op0=ALU.add, op1=ALU.add)

        # ---- pass A2: v + gate ----
        with ExitStack() as actx:
            wp2 = actx.enter_context(tc.tile_pool(name="wp2", bufs=1))
            xp2 = actx.enter_context(tc.tile_pool(name="xp2", bufs=2))
            cp2 = actx.enter_context(tc.tile_pool(name="cp2", bufs=2))
            pa2 = actx.enter_context(tc.tile_pool(name="pa2", bufs=3, space="PSUM"))
            conv_silu2 = make_conv_silu(cp2)

            w_vg_sb = wp2.tile([128, KS, 1536], BF16)
            nc.sync.dma_start(out=w_vg_sb, in_=w_vg[:].rearrange("(k p) c -> p k c", p=128))

            for tt in range(NT):
                t0 = tt * TB
                x_t = xp2.tile([128, KS, TB], BF16, name="x_t2")
                nc.sync.dma_start(out=x_t, in_=xt[:, t0:t0 + TB].rearrange("(k p) t -> p k t", p=128))

                for u in range(3):
                    vsl = cp2.tile([128, 2, TB], BF16, tag="vsl", name="vsl")
                    for ft in range(2):
                        ps = pa2.tile([128, TB], F32, tag="proj", name="ps_v")
                        col = u * 512 + ft * 128
                        for k in range(KS):
                            nc.tensor.matmul(ps, w_vg_sb[:, k, col:col + 128],
                                             x_t[:, k, :], start=(k == 0), stop=(k == KS - 1))
                        conv_silu2(ps, ("v", u, ft), 32 + u * 8 + ft * 4,
                                   out=vsl[:, ft, :])
                    nc.sync.dma_start(
                        out=v_dr[u * 256:(u + 1) * 256, t0:t0 + TB]
                        .rearrange("(f p) t -> p f t", p=128),
                        in_=vsl)

                for st in range(TB // 128):
                    gl768 = cp2.tile([128, 768], BF16, tag="gl768", name="gl768")
                    for u in range(3):
                        ps = pa2.tile([128, 256], F32, tag="gate", name="ps_g")
                        col = u * 512 + 256
                        for k in range(KS):
                            nc.tensor.matmul(ps, x_t[:, k, st * 128:(st + 1) * 128],
                                             w_vg_sb[:, k, col:col + 256],
                                             start=(k == 0), stop=(k == KS - 1))
                        ge = cp2.tile([128, 256], BF16, tag="ge", name="ge")
                        nc.scalar.activation(out=ge, in_=ps, func=AF.Exp, scale=-1.0)
                        nc.gpsimd.tensor_scalar_add(ge, ge, 1.0)
                        with nc.allow_low_precision("sigmoid denom bf16"):
                            nc.vector.reciprocal(ge, ge)
                        nc.vector.tensor_mul(gl768[:, u * 256:(u + 1) * 256], ps, ge)
                    nc.sync.dma_start(
                        out=gate_dr[t0 + st * 128:t0 + (st + 1) * 128, :],
                        in_=gl768)

        # ================= PHASE B + C =================
        with ExitStack() as bctx:
            bp = bctx.enter_context(tc.tile_pool(name="bp", bufs=4))
            ep = bctx.enter_context(tc.tile_pool(name="ep", bufs=4))
            vp = bctx.enter_context(tc.tile_pool(name="vp", bufs=3))
            pm = bctx.enter_context(tc.tile_pool(name="pm", bufs=3, space="PSUM"))
            pt = bctx.enter_context(tc.tile_pool(name="pt", bufs=2, space="PSUM"))
            pc = bctx.enter_context(tc.tile_pool(name="pc", bufs=3, space="PSUM"))

            for c in range(NCH):
                a0 = c * C
                sl = slice(a0, a0 + C)
                # shared per-chunk: transpose [G;beta] -> cols
                gb = ep.tile([97, C], F32, tag="gb")
                nc.gpsimd.memset(gb, 0.0)
                nc.gpsimd.tensor_copy(out=gb[0:1, :], in_=Grows[0:1, sl])
                nc.gpsimd.tensor_copy(out=gb[32:33, :], in_=Grows[32:33, sl])
                nc.gpsimd.tensor_copy(out=gb[64:65, :], in_=brows[0:1, sl])
                nc.gpsimd.tensor_copy(out=gb[96:97, :], in_=brows[32:33, sl])
                ps_t = pm.tile([128, 97], F32, tag="pm")
                nc.tensor.transpose(ps_t, gb, idf_sb[0:97, 0:97])
                cols = ep.tile([128, 97], F32, tag="cols")
                nc.scalar.copy(out=cols, in_=ps_t)

                slot = {}
                for s in range(2):
                    # q/k chunk slices (one DMA: j = qk*2 + ft)
                    qks = vp.tile([128, 4, C], BF16, tag="qks")
                    nc.sync.dma_start(out=qks, in_=qk_dr[s * 512:(s + 1) * 512, sl]
                                      .rearrange("(j p) t -> p j t", p=128))

                    gcol = cols[:, 32 * s:32 * s + 1]
                    bcol = cols[:, 64 + 32 * s:65 + 32 * s]
                    # G_C broadcast col + derived scalars
                    ps_gc = pm.tile([128, 1], F32, tag="pm")
                    R = 32 * s
                    nc.tensor.matmul(ps_gc, ones_f_row[R:R + 1, :],
                                     Grows[R:R + 1, a0 + C - 1:a0 + C],
                                     start=True, stop=True)
                    gcc = ep.tile([128, 1], F32, tag="gcc")
                    nc.scalar.copy(out=gcc, in_=ps_gc)
                    eGC = ep.tile([128, 1], F32, tag="eGC")
                    nc.scalar.activation(out=eGC, in_=gcc, func=AF.Exp)
                    ktsc = ep.tile([128, 1], F32, tag="ktsc")
                    nc.scalar.activation(out=ktsc, in_=gcol, func=AF.Exp,
                                         scale=-1.0, bias=gcc)
                    nc.vector.tensor_mul(ktsc, ktsc, bcol)
                    egcol = ep.tile([128, 1], F32, tag="egcol")
                    nc.scalar.activation(out=egcol, in_=gcol, func=AF.Exp)

                    # replicated rows
                    ps_gr = pm.tile([128, C], F32, tag="pm")
                    nc.tensor.matmul(ps_gr, ones_f_row[R:R + 1, :], Grows[R:R + 1, sl],
                                     start=True, stop=True)

                    # E matrices (fused): strict-lower for Mn, incl-upper for
                    # MnT (diag zeroed via mask) and PT
                    ps_brn = pm.tile([128, C], F32, tag="pm")
                    nc.tensor.matmul(ps_brn, ones_f_row[R:R + 1, :],
                                     brows_neg[R:R + 1, sl], start=True, stop=True)
                    dd = ep.tile([128, C], F32, tag="dd")
                    nc.vector.scalar_tensor_tensor(out=dd, in0=ps_gr, scalar=gcol,
                                                   in1=msk_sb[:, 0:128],
                                                   op0=ALU.subtract, op1=ALU.add)
                    es = ep.tile([128, C], F32, tag="es")
                    nc.scalar.activation(out=es, in_=dd, func=AF.Exp, scale=-1.0)
                    dd3 = ep.tile([128, C], F32, tag="dd3")
                    nc.vector.scalar_tensor_tensor(out=dd3, in0=ps_gr, scalar=gcol,
                                                   in1=msk_sb[:, 128:256],
                                                   op0=ALU.subtract, op1=ALU.add)
                    eupi = ep.tile([128, C], F32, tag="eupi")
                    nc.scalar.activation(out=eupi, in_=dd3, func=AF.Exp)

                    esb = ep.tile([128, C], F32, tag="esb")
                    nc.vector.tensor_mul(esb, es, ps_brn)       # -beta_j e^{Gi-Gj} strict-low
                    eupn = ep.tile([128, C], F32, tag="eupn")   # -beta_i e^{Gj-Gi} strict-up
                    nc.vector.scalar_tensor_tensor(out=eupn, in0=eupi, scalar=bcol,
                                                   in1=msk_sb[:, 256:384],
                                                   op0=ALU.mult, op1=ALU.mult)
                    nc.vector.tensor_scalar_mul(eupi, eupi, bcol)

                    # KK / KQ
                    ps_kk = pm.tile([128, C], F32, tag="pm")
                    for ft in range(2):
                        nc.tensor.matmul(ps_kk, qks[:, 2 + ft, :], qks[:, 2 + ft, :],
                                         start=(ft == 0), stop=(ft == 1))
                    ps_kq = pm.tile([128, C], F32, tag="pm")
                    for ft in range(2):
                        nc.tensor.matmul(ps_kq, qks[:, 2 + ft, :], qks[:, ft, :],
                                         start=(ft == 0), stop=(ft == 1))
                    Mn = bp.tile([128, C], BF16, tag="Mn")
                    nc.vector.tensor_mul(Mn, esb, ps_kk)
                    MnT = bp.tile([128, C], BF16, tag="MnT")
                    nc.vector.tensor_mul(MnT, eupn, ps_kk)
                    PT = bp.tile([128, C], BF16, tag="PT")
                    nc.vector.tensor_mul(PT, eupi, ps_kq)

                    # T-chain: A = Tmat^T
                    A = bp.tile([128, C], BF16, tag="A0")
                    nc.vector.tensor_add(A, idb_sb, MnT)
                    ps_x = pm.tile([128, C], F32, tag="pm")
                    nc.tensor.matmul(ps_x, MnT, Mn, start=True, stop=True)
                    X = bp.tile([128, C], BF16, tag="X")
                    nc.scalar.copy(out=X, in_=ps_x)
                    ps_x2 = pm.tile([128, C], F32, tag="pm")
                    nc.tensor.matmul(ps_x2, Mn, MnT, start=True, stop=True)
                    XT = bp.tile([128, C], BF16, tag="XT")
                    nc.scalar.copy(out=XT, in_=ps_x2)
                    for lvl in range(NLVL - 1):
                        ps_a = pm.tile([128, C], F32, tag="pm")
                        nc.tensor.matmul(ps_a, X, A, start=True, stop=True)
                        A2 = bp.tile([128, C], BF16, tag=f"A{lvl + 1}")
                        nc.vector.tensor_add(A2, ps_a, A)
                        A = A2
                        if lvl < NLVL - 2:
                            ps_y = pm.tile([128, C], F32, tag="pm")
                            nc.tensor.matmul(ps_y, XT, X, start=True, stop=True)
                            ps_y2 = pm.tile([128, C], F32, tag="pm")
                            nc.tensor.matmul(ps_y2, X, XT, start=True, stop=True)
                            X = bp.tile([128, C], BF16, tag="X")
                            nc.scalar.copy(out=X, in_=ps_y)
                            XT = bp.tile([128, C], BF16, tag="XT")
                            nc.scalar.copy(out=XT, in_=ps_y2)

                    # K time-major + scaled variants
                    ktm = bp.tile([128, 256], BF16, tag="ktm")
                    for ft in range(2):
                        ps_tr = pt.tile([128, 128], BF16, tag="tr")
                        nc.tensor.transpose(ps_tr, qks[:, 2 + ft, :], idb_sb)
                        nc.scalar.copy(out=ktm[:, ft * 128:(ft + 1) * 128], in_=ps_tr)
                    kgn = bp.tile([128, 256], BF16, tag="kgn")
                    nc.vector.tensor_scalar(out=kgn, in0=ktm, scalar1=egcol,
                                            scalar2=-1.0, op0=ALU.mult, op1=ALU.mult)
                    ktl = bp.tile([128, 256], BF16, tag="ktl")
                    nc.vector.tensor_scalar_mul(ktl, ktm, ktsc)
                    tkgn = bp.tile([128, 256], BF16, tag="tkgn")
                    for ft in range(2):
                        ps_k = pm.tile([128, C], F32, tag="pm")
                        nc.tensor.matmul(ps_k, kgn[:, ft * 128:(ft + 1) * 128], A,
                                         start=True, stop=True)
                        nc.scalar.copy(out=tkgn[:, ft * 128:(ft + 1) * 128], in_=ps_k)
                    egr = ep.tile([1, C], BF16, tag="egr")
                    nc.scalar.activation(out=egr, in_=Grows[R:R + 1, sl], func=AF.Exp)
                    ps_eg = pm.tile([128, C], F32, tag="pm")
                    nc.tensor.matmul(ps_eg, ones_b_row[0:1, :], egr,
                                     start=True, stop=True)
                    qgt = bp.tile([128, 256], BF16, tag="qgt")
                    for ft in range(2):
                        nc.vector.tensor_mul(qgt[:, ft * 128:(ft + 1) * 128],
                                             qks[:, ft, :], ps_eg)
                    slot[s] = dict(A=A, PT=PT, tkgn=tkgn, qgt=qgt, ktl=ktl, eGC=eGC)

                # ---- per-unit chain ----
                for u in range(3):
                    s = SU[u]
                    sd = slot[s]
                    vfa = vp.tile([128, 2, C], BF16, tag="vfa")
                    nc.sync.dma_start(out=vfa, in_=v_dr[u * 256:(u + 1) * 256, sl]
                                      .rearrange("(f p) t -> p f t", p=128))
                    vtm = vp.tile([128, 256], BF16, tag="vtm")
                    for ft in range(2):
                        ps_tr = pt.tile([128, 128], BF16, tag="tr")
                        nc.tensor.transpose(ps_tr, vfa[:, ft, :], idb_sb)
                        nc.scalar.copy(out=vtm[:, ft * 128:(ft + 1) * 128], in_=ps_tr)

                    ps_u = pc.tile([128, DVU], F32, tag="pc")
                    nc.tensor.matmul(ps_u, sd["A"], vtm, start=True, stop=(c == 0))
                    if c > 0:
                        for ft in range(2):
                            nc.tensor.matmul(ps_u, sd["tkgn"][:, ft * 128:(ft + 1) * 128],
                                             S_b[u][:, ft, :], start=False, stop=(ft == 1))
                    u_sb = vp.tile([128, DVU], BF16, tag="usb")
                    nc.vector.tensor_copy(out=u_sb, in_=ps_u)

                    ps_o = pc.tile([128, DVU], F32, tag="pc")
                    if c > 0:
                        for ft in range(2):
                            nc.tensor.matmul(ps_o, sd["qgt"][:, ft * 128:(ft + 1) * 128],
                                             S_b[u][:, ft, :], start=(ft == 0), stop=False)
                    nc.tensor.matmul(ps_o, sd["PT"], u_sb, start=(c == 0), stop=True)
                    obf = vp.tile([128, DVU], BF16, tag="obf")
                    nc.vector.tensor_copy(out=obf, in_=ps_o)
                    nc.sync.dma_start(out=o_dr[sl, u * 256:(u + 1) * 256], in_=obf)
                    trash = vp.tile([128, DVU], BF16, tag="trash")
                    nc.scalar.activation(out=trash, in_=ps_o, func=AF.Square,
                                         accum_out=ssq[:, u * NCH + c:u * NCH + c + 1])

                    for ft in range(2):
                        ps_s = pc.tile([128, DVU], F32, tag="pc")
                        nc.tensor.matmul(ps_s, sd["ktl"][:, ft * 128:(ft + 1) * 128],
                                         u_sb, start=True, stop=True)
                        nc.vector.scalar_tensor_tensor(
                            out=S_f[u][:, ft, :], in0=S_f[u][:, ft, :],
                            scalar=sd["eGC"], in1=ps_s, op0=ALU.mult, op1=ALU.add)
                    nc.gpsimd.tensor_copy(out=S_b[u], in_=S_f[u])

        # ---- ssq pair exchange ----
        nc.sync.dma_start(out=ssq_cc_in[:], in_=ssq[:, 2 * NCH:3 * NCH])
        nc.gpsimd.collective_compute(
            "AllReduce", mybir.AluOpType.add,
            replica_groups=[[0, 1], [2, 3], [4, 5], [6, 7]],
            ins=[ssq_cc_in[:]], outs=[ssq_cc_out[:]])
        nc.sync.dma_start(out=ssq2x, in_=ssq_cc_out[:])

        # ================= PHASE C2 =================
        with ExitStack() as cctx:
            np_ = cctx.enter_context(tc.tile_pool(name="np", bufs=3))
            pn = cctx.enter_context(tc.tile_pool(name="pn", bufs=2, space="PSUM"))
            for c in range(NCH):
                a0 = c * C
                sl = slice(a0, a0 + C)
                totA = np_.tile([128, 1], F32, tag="totA")
                nc.vector.tensor_add(totA, ssq[:, c:c + 1], ssq[:, NCH + c:NCH + c + 1])
                rsqA = np_.tile([128, 1], F32, tag="rsqA")
                nc.scalar.activation(out=rsqA, in_=totA, func=AF.Ln,
                                     scale=float(1.0 / DV), bias=cst_sb[:, 61:62])
                nc.scalar.activation(out=rsqA, in_=rsqA, func=AF.Exp, scale=-0.5)
                rsqB = np_.tile([128, 1], F32, tag="rsqB")
                nc.scalar.activation(out=rsqB, in_=ssq2x[:, c:c + 1], func=AF.Ln,
                                     scale=float(1.0 / DV), bias=cst_sb[:, 61:62])
                nc.scalar.activation(out=rsqB, in_=rsqB, func=AF.Exp, scale=-0.5)
                ot768 = np_.tile([128, 768], BF16, tag="ot768")
                nc.sync.dma_start(out=ot768, in_=o_dr[sl, :])
                gt768 = np_.tile([128, 768], BF16, tag="gt768")
                nc.sync.dma_start(out=gt768, in_=gate_dr[sl, :])
                for u in range(3):
                    rsq = rsqA if u < 2 else rsqB
                    otn = np_.tile([128, DVU], BF16, tag="otn")
                    nc.vector.scalar_tensor_tensor(
                        out=otn, in0=ot768[:, u * 256:(u + 1) * 256], scalar=rsq,
                        in1=gt768[:, u * 256:(u + 1) * 256],
                        op0=ALU.mult, op1=ALU.mult)
                    otr2 = np_.tile([128, 2, 128], BF16, tag="otr2")
                    for ft in range(2):
                        ps_tr = pn.tile([128, 128], BF16, tag="tr")
                        nc.tensor.transpose(ps_tr, otn[:, ft * 128:(ft + 1) * 128], idb_sb)
                        nc.vector.tensor_copy(out=otr2[:, ft, :], in_=ps_tr)
                    nc.sync.dma_start(
                        out=ot_dr[u * 256:(u + 1) * 256, sl]
                        .rearrange("(f p) t -> p f t", p=128),
                        in_=otr2)

        # ================= PHASE D =================
        with ExitStack() as dctx:
            dp = dctx.enter_context(tc.tile_pool(name="dp", bufs=3))
            wop = dctx.enter_context(tc.tile_pool(name="wop", bufs=1))
            pd = dctx.enter_context(tc.tile_pool(name="pd", bufs=3, space="PSUM"))
            w_o_sb = wop.tile([128, 6, HID], BF16)
            nc.sync.dma_start(out=w_o_sb, in_=w_o[:].rearrange("(r p) n -> p r n", p=128))
            for st4 in range(4):
                for ti in range(8):
                    rows = st4 * 1024 + ti * 128
                    otl = dp.tile([128, 6, 128], BF16, tag="otl")
                    nc.sync.dma_start(out=otl, in_=ot_dr[:, rows:rows + 128]
                                      .rearrange("(r p) t -> p r t", p=128))
                    pob = dp.tile([128, 2048], BF16, tag="pob")
                    for nt_ in range(4):
                        ps = pd.tile([128, 512], F32, tag="pd")
                        for r in range(6):
                            nc.tensor.matmul(ps, otl[:, r, :],
                                             w_o_sb[:, r, nt_ * 512:(nt_ + 1) * 512],
                                             start=(r == 0), stop=(r == 5))
                        eng = nc.scalar if nt_ % 2 == 0 else nc.vector
                        if nt_ % 2 == 0:
                            nc.scalar.copy(out=pob[:, nt_ * 512:(nt_ + 1) * 512], in_=ps)
                        else:
                            nc.vector.tensor_copy(out=pob[:, nt_ * 512:(nt_ + 1) * 512], in_=ps)
                    nc.sync.dma_start(out=po_st[st4][ti * 128:(ti + 1) * 128, :],
                                      in_=pob)
                nc.gpsimd.collective_compute(
                    "ReduceScatter", mybir.AluOpType.add,
                    replica_groups=[[0, 1, 2, 3], [4, 5, 6, 7]],
                    ins=[po_st[st4][:]], outs=[rs_st[st4][:]])
                for r2 in range(2):
                    rsb = dp.tile([128, HID], BF16, tag="rsb")
                    nc.sync.dma_start(out=rsb, in_=rs_st[st4][r2 * 128:(r2 + 1) * 128, :])
                    rsf = dp.tile([128, HID], F32, tag="rsf")
                    nc.vector.tensor_copy(out=rsf, in_=rsb)
                    nc.sync.dma_start(out=out[st4 * 256 + r2 * 128:
                                              st4 * 256 + (r2 + 1) * 128, :], in_=rsf)

    import concourse.bacc as bacc_mod
    _orig_gat = bacc_mod.get_activation_tables
    def _pin_tables(arch):
        t = _orig_gat(arch)
        return {k: (v if k == "natural_log_exp_and_others" else set())
                for k, v in t.items()}
    bacc_mod.get_activation_tables = _pin_tables
    try:
        nc.compile()
    finally:
        bacc_mod.get_activation_tables = _orig_gat
    _BUILD_CACHE["nc"] = nc
    return nc


def _prep_core_inputs(ins, core):
    """Pack per-core input arrays. ins: dict of fp32 numpy arrays."""
    b, g = core // 4, core % 4
    hA, hB, dvhB = GROUPS[g]
    units = [(hA, 0), (hA, 1), (hB, dvhB)]
    heads = [hA, hB]

    x = np.asarray(ins["hidden_states"], np.float32)[b]        # [T, HID]
    xt = np.ascontiguousarray(x.T).astype(BF)                  # [HID, T]

    w_qk = np.zeros((HID, 1024), np.float32)
    w_ab = np.zeros((HID, 128), np.float32)
    for s, h in enumerate(heads):
        w_qk[:, s * 512:s * 512 + 256] = ins["Wq"][h * DK:(h + 1) * DK].T
        w_qk[:, s * 512 + 256:s * 512 + 512] = ins["Wk"][h * DK:(h + 1) * DK].T
        w_ab[:, 32 * s] = ins["Wa"][h]
        w_ab[:, 64 + 32 * s] = ins["Wb"][h]

    w_vg = np.zeros((HID, 1536), np.float32)
    w_o = np.zeros((768, HID), np.float32)
    for u, (h, dvh) in enumerate(units):
        r = slice(h * DV + dvh * 256, h * DV + dvh * 256 + 256)
        w_vg[:, u * 512:u * 512 + 256] = ins["Wv"][r].T
        w_vg[:, u * 512 + 256:u * 512 + 512] = ins["Wg"][r].T
        nw = ins["norm_w"][dvh * 256:(dvh + 1) * 256]
        w_o[u * 256:(u + 1) * 256, :] = (ins["Wo"][:, r].T * nw[:, None])

    cst = np.zeros((128, 64), np.float32)
    for s, h in enumerate(heads):
        for qk, cw in ((0, ins["conv_wq"]), (1, ins["conv_wk"])):
            for ft in range(2):
                for j in range(4):
                    cst[:, s * 16 + qk * 8 + ft * 4 + j] = \
                        cw[h * DK + ft * 128:h * DK + (ft + 1) * 128, j]
    for u, (h, dvh) in enumerate(units):
        for ft in range(2):
            for j in range(4):
                cst[:, 32 + u * 8 + ft * 4 + j] = \
                    ins["conv_wv"][h * DV + dvh * 256 + ft * 128:
                                   h * DV + dvh * 256 + (ft + 1) * 128, j]
    for s, h in enumerate(heads):
        cst[32 * s, 56] = ins["dt_bias"][h]
        cst[32 * s, 57] = -np.exp(ins["A_log"][h])
    cst[:, 58] = 1.0
    cst[:, 59] = DK * L2_EPS
    cst[:, 60] = L2_EPS
    cst[:, 61] = NORM_EPS

    ii, jj = np.mgrid[0:128, 0:128]
    msk = np.zeros((128, 384), np.float32)
    msk[:, 0:128] = np.where(jj >= ii, 1e9, 0.0)       # pre-exp(-x) strict-lower
    msk[:, 128:256] = np.where(jj < ii, -1e9, 0.0)     # pre-exp(+x) incl-upper
    msk[:, 256:384] = np.where(jj > ii, -1.0, 0.0)     # -(strict upper 0/1)

    idf = np.eye(128, dtype=np.float32)

    return {
        "xt": np.ascontiguousarray(xt),
        "w_qk": np.ascontiguousarray(w_qk.astype(BF)),
        "w_ab": np.ascontiguousarray(w_ab.astype(BF)),
        "w_vg": np.ascontiguousarray(w_vg.astype(BF)),
        "w_o": np.ascontiguousarray(w_o.astype(BF)),
        "cst": cst,
        "msk": msk,
        "idf": idf,
        "idb": np.ascontiguousarray(idf.astype(BF)),
    }


LAST_RESULTS = None


def _run_device(ins, trace=False):
    global LAST_RESULTS
    from concourse.bass_utils import run_bass_kernel_spmd
    nc = _build()
    in_maps = [_prep_core_inputs(ins, c) for c in range(N_CORES)]
    res = run_bass_kernel_spmd(nc, in_maps, list(range(N_CORES)), trace=trace)
    LAST_RESULTS = res
    B = 2
    full = np.empty((B, T, HID), np.float32)
    for core in range(N_CORES):
        b, g = core // 4, core % 4
        o = res.results[core]["out"]                # [1024, HID]
        for st4 in range(4):
            full[b, st4 * 1024 + g * 256: st4 * 1024 + (g + 1) * 256] = \
                o[st4 * 256:(st4 + 1) * 256]
    return full


def kernel(**inputs):
    ins = {k: np.asarray(v, np.float32) for k, v in inputs.items()}
    return _run_device(ins)


# revision 24
# speedup vs baseline: 4.4833x; 3.8566x over previous
"""GatedDeltaNet on 8 Trainium2 NeuronCores (Bass/Tile).

Sharding: 2 batches x 4 cores. Each core owns 3 units of (head, dv-half) of
its batch: slot A = a full head (both dv halves), slot B = one half of a
shared head (pair-exchanged with the neighbor core for the RMS norm).

Per-core pipeline (one NEFF, all cores run the same program):
  A: projections from feature-major x^T (bf16 matmuls), causal dwconv+SiLU
     (feature-major, scalar_tensor_tensor taps), l2norm (ones-matmul
     partition reduction), beta/g rows, per-chunk g-cumsum (tensor scan).
  B: per chunk (C=128) per head-slot: decay matrices E via ACT exp of
     G_i - G_j, M = -(KK^T o E), Tmat^T = ((I+M)^{-1})^T via nilpotent
     Neumann doubling (bf16 matmuls), P^T, K~ (chunk-local), TKg^T, Qg^T.
  C: sequential chunk chain per unit: u = Tmat(v - Kg S), o = Qg S + P u,
     S = exp(G_C) S + K~^T u (fp32 master state, bf16 matmul shadow).
  C2: RMS norm (pair AllReduce for the straddling head's sum-of-squares),
     SiLU gate, PE-transpose o -> feature-major.
  D: output projection (norm_w folded into Wo) + grouped ReduceScatter.
"""
import numpy as np
import ml_dtypes

BF = ml_dtypes.bfloat16

# model dims
H, DK, DV, HID, CONV = 6, 256, 512, 2048, 4
NORM_EPS = 1e-5
L2_EPS = 1e-6

# kernel config
N_CORES = 8
T = 4096            # per batch
TB = 512            # phase-A time tile
NT = T // TB        # 8
KS = HID // 128     # 16 contraction slices
C = 128             # chunk length
NCH = T // C        # 32
DVU = 256           # dv per unit
NLVL = 2            # Neumann factors (I+Mn)(I+Mn^2)
SU = (0, 0, 1)      # unit -> head slot

# per-core unit tables: group g -> (headA, headB, dvh of unit2)
GROUPS = [
    (0, 1, 0),
    (2, 1, 1),
    (3, 4, 0),
    (5, 4, 1),
]

_BUILD_CACHE = {}


def _build():
    if "nc" in _BUILD_CACHE:
        return _BUILD_CACHE["nc"]
    import concourse.bass as bass  # noqa
    import concourse.bacc as bacc
    import concourse.mybir as mybir
    import concourse.tile as tile
    from contextlib import ExitStack

    F32 = mybir.dt.float32
    BF16 = mybir.dt.bfloat16
    AF = mybir.ActivationFunctionType
    ALU = mybir.AluOpType

    nc = bacc.Bacc("TRN2", target_bir_lowering=False, debug=False,
                   num_devices=N_CORES)

    xt = nc.dram_tensor("xt", [HID, T], BF16, kind="ExternalInput")
    w_qk = nc.dram_tensor("w_qk", [HID, 1024], BF16, kind="ExternalInput")
    w_ab = nc.dram_tensor("w_ab", [HID, 128], BF16, kind="ExternalInput")
    w_v = nc.dram_tensor("w_v", [HID, 768], BF16, kind="ExternalInput")
    w_g = nc.dram_tensor("w_g", [HID, 768], BF16, kind="ExternalInput")
    w_o = nc.dram_tensor("w_o", [768, HID], BF16, kind="ExternalInput")
    cst = nc.dram_tensor("cst", [128, 64], F32, kind="ExternalInput")
    msk = nc.dram_tensor("msk", [128, 384], F32, kind="ExternalInput")
    idf = nc.dram_tensor("idf", [128, 128], F32, kind="ExternalInput")
    idb = nc.dram_tensor("idb", [128, 128], BF16, kind="ExternalInput")
    out = nc.dram_tensor("out", [1024, HID], F32, kind="ExternalOutput")

    with tile.TileContext(nc) as tc, ExitStack() as ctx:
        pers = ctx.enter_context(tc.tile_pool(name="pers", bufs=1))
        dram = ctx.enter_context(tc.tile_pool(name="dram", bufs=1, space="DRAM"))

        # persistent SBUF
        brows = pers.tile([64, T], F32)
        brows_neg = pers.tile([64, T], F32)
        Grows = pers.tile([64, T], F32)
        ssq = pers.tile([128, 3 * NCH], F32)
        ssq2x = pers.tile([128, NCH], F32)
        S_f = [pers.tile([128, 2, DVU], F32, name=f"S_f{u}") for u in range(3)]
        S_b = [pers.tile([128, 2, DVU], BF16, name=f"S_b{u}") for u in range(3)]
        cst_sb = pers.tile([128, 64], F32)
        msk_sb = pers.tile([128, 384], F32)
        idf_sb = pers.tile([128, 128], F32)
        idb_sb = pers.tile([128, 128], BF16)
        ones_f_row = pers.tile([33, 128], F32)
        ones_b_row = pers.tile([33, 128], BF16)
        ones_b_col = pers.tile([128, 1], BF16)

        nc.sync.dma_start(out=cst_sb, in_=cst[:])
        nc.sync.dma_start(out=msk_sb, in_=msk[:])
        nc.sync.dma_start(out=idf_sb, in_=idf[:])
        nc.sync.dma_start(out=idb_sb, in_=idb[:])
        nc.vector.memset(ones_f_row, 1.0)
        nc.vector.memset(ones_b_row, 1.0)
        nc.vector.memset(ones_b_col, 1.0)
        for u in range(3):
            nc.vector.memset(S_f[u], 0.0)
            nc.vector.memset(S_b[u], 0.0)

        # DRAM scratch
        qk_drs = [dram.tile([1024, TB], BF16, name=f"qkd{i}") for i in range(NT)]
        v_drs = [dram.tile([768, TB], BF16, name=f"vd{i}") for i in range(NT)]
        gate_drs = [dram.tile([TB, 768], BF16, name=f"gd{i}") for i in range(NT)]
        o_drs = [dram.tile([C, 256], BF16, name=f"od{i}") for i in range(NCH)]
        o2_drs = [dram.tile([C, 768], BF16, name=f"o2d{i}") for i in range(NCH)]
        ssq_cc_in = [dram.tile([128, NCH // 2], F32, name=f"ssq_in{i}") for i in range(2)]
        ssq_cc_out = [dram.tile([128, NCH // 2], F32, name=f"ssq_out{i}") for i in range(2)]
        po_st = [dram.tile([512, HID], BF16, name=f"po_st{i}") for i in range(8)]
        rs_st = [dram.tile([128, HID], BF16, name=f"rs_st{i}") for i in range(8)]

        # conv halo carries (projection values of the last 3 steps)
        carries = {}
        for s in range(2):
            for qk in range(2):
                for ft in range(2):
                    carries[("qk", s, qk, ft)] = pers.tile([128, 3], BF16, name=f"cr_qk{s}{qk}{ft}")
        for u in range(3):
            for ft in range(2):
                carries[("v", u, ft)] = pers.tile([128, 3], BF16, name=f"cr_v{u}{ft}")
        for t_ in carries.values():
            nc.vector.memset(t_, 0.0)

        # ================= PHASE A =================
        def silu_from(cp, src_ap, out_ap, tag):
            """out = src * sigmoid(src), sigmoid via exp/ln/exp (one ACT table)."""
            e1 = cp.tile([128, TB], F32, tag=tag + "e1", name="e1")
            nc.scalar.activation(out=e1, in_=src_ap, func=AF.Exp, scale=-1.0)
            nc.scalar.activation(out=e1, in_=e1, func=AF.Ln, bias=1.0)
            sg = cp.tile([128, TB], BF16, tag=tag + "sg", name="sg")
            nc.scalar.activation(out=sg, in_=e1, func=AF.Exp, scale=-1.0)
            nc.vector.tensor_mul(out_ap, src_ap, sg)

        def make_conv_silu(cp):
            def conv_silu(ps, stream, wcol0, out=None):
                """ps: PSUM [128, TB] projection -> causal conv + SiLU, bf16."""
                pbuf = cp.tile([128, TB + 3], BF16, tag="pbuf", name="pbuf")
                nc.gpsimd.tensor_copy(out=pbuf[:, 0:3], in_=carries[stream])
                nc.scalar.copy(out=pbuf[:, 3:TB + 3], in_=ps)
                nc.gpsimd.tensor_copy(out=carries[stream], in_=pbuf[:, TB:TB + 3])
                cv = cp.tile([128, TB], BF16, tag="cv", name="cv")
                nc.vector.tensor_scalar_mul(cv, pbuf[:, 0:TB], cst_sb[:, wcol0:wcol0 + 1])
                for j in range(1, 4):
                    nc.vector.scalar_tensor_tensor(
                        out=cv, in0=pbuf[:, j:j + TB],
                        scalar=cst_sb[:, wcol0 + j:wcol0 + j + 1], in1=cv,
                        op0=ALU.mult, op1=ALU.add)
                sl_ = out
                if sl_ is None:
                    sl_ = cp.tile([128, TB], BF16, tag=f"sl{stream[-1]}",
                                  name=f"sl{stream[-1]}")
                silu_from(cp, cv, sl_, "vs")
                return sl_
            return conv_silu

        # ---- pass A1: q/k + a/b ----
        with ExitStack() as actx:
            wp = actx.enter_context(tc.tile_pool(name="wp", bufs=1))
            xp = actx.enter_context(tc.tile_pool(name="xp", bufs=2))
            cp = actx.enter_context(tc.tile_pool(name="cp", bufs=2))
            pa = actx.enter_context(tc.tile_pool(name="pa", bufs=2, space="PSUM"))
            pb_ = actx.enter_context(tc.tile_pool(name="pb", bufs=1, space="PSUM"))
            conv_silu = make_conv_silu(cp)

            w_qk_sb = wp.tile([128, KS, 1024], BF16)
            nc.sync.dma_start(out=w_qk_sb, in_=w_qk[:].rearrange("(k p) c -> p k c", p=128))
            w_ab_sb = wp.tile([128, KS, 128], BF16)
            nc.sync.dma_start(out=w_ab_sb, in_=w_ab[:].rearrange("(k p) c -> p k c", p=128))

            for tt in range(NT):
                t0 = tt * TB
                x_t = xp.tile([128, KS, TB], BF16, name="x_t")
                nc.sync.dma_start(out=x_t, in_=xt[:, t0:t0 + TB].rearrange("(k p) t -> p k t", p=128))

                for s in range(2):
                    qn4 = cp.tile([128, 4, TB], BF16, tag="qn4", name="qn4")
                    for qk in range(2):
                        sls = []
                        ps_ss = pb_.tile([1, TB], F32, tag="ss", name="ps_ss")
                        for ft in range(2):
                            ps = pa.tile([128, TB], F32, tag="proj", name="ps_qk")
                            col = s * 512 + qk * 256 + ft * 128
                            for k in range(KS):
                                nc.tensor.matmul(ps, w_qk_sb[:, k, col:col + 128],
                                                 x_t[:, k, :], start=(k == 0), stop=(k == KS - 1))
                            stream = ("qk", s, qk, ft)
                            wcol0 = s * 16 + qk * 8 + ft * 4
                            pbuf = cp.tile([128, TB + 3], BF16, tag="pbuf", name="pbuf")
                            nc.gpsimd.tensor_copy(out=pbuf[:, 0:3], in_=carries[stream])
                            nc.scalar.copy(out=pbuf[:, 3:TB + 3], in_=ps)
                            nc.gpsimd.tensor_copy(out=carries[stream], in_=pbuf[:, TB:TB + 3])
                            cv = cp.tile([128, TB], BF16, tag="cv", name="cv")
                            nc.vector.tensor_scalar_mul(cv, pbuf[:, 0:TB],
                                                        cst_sb[:, wcol0:wcol0 + 1])
                            for j in range(1, 4):
                                nc.vector.scalar_tensor_tensor(
                                    out=cv, in0=pbuf[:, j:j + TB],
                                    scalar=cst_sb[:, wcol0 + j:wcol0 + j + 1], in1=cv,
                                    op0=ALU.mult, op1=ALU.add)
                            sl_ = cp.tile([128, TB], BF16, tag=f"sl{ft}", name="sl")
                            silu_from(cp, cv, sl_, f"cs{ft}")
                            sq = cp.tile([128, TB], BF16, tag="sq", name="sq")
                            nc.vector.tensor_mul(sq, sl_, sl_)
                            nc.tensor.matmul(ps_ss, ones_b_col, sq,
                                             start=(ft == 0), stop=(ft == 1))
                            sls.append(sl_)
                        rs_row = cp.tile([1, TB], F32, tag="rsr", name="rs_row")
                        if qk == 0:  # q: 1/sqrt(DK*(ss+eps))
                            nc.scalar.activation(out=rs_row, in_=ps_ss, func=AF.Ln,
                                                 scale=float(DK), bias=cst_sb[0:1, 59:60])
                        else:
                            nc.scalar.activation(out=rs_row, in_=ps_ss, func=AF.Ln,
                                                 scale=1.0, bias=cst_sb[0:1, 60:61])
                        nc.scalar.activation(out=rs_row, in_=rs_row, func=AF.Exp,
                                             scale=-0.5)
                        rs_bf = cp.tile([1, TB], BF16, tag="rsb", name="rs_bf")
                        nc.vector.tensor_copy(out=rs_bf, in_=rs_row)
                        ps_bc = pb_.tile([128, TB], F32, tag="bc", name="ps_bc")
                        nc.tensor.matmul(ps_bc, ones_b_row[0:1, :], rs_bf, start=True, stop=True)
                        for ft in range(2):
                            nc.vector.tensor_mul(qn4[:, qk * 2 + ft, :], sls[ft], ps_bc)
                    nc.sync.dma_start(
                        out=qk_drs[tt][s * 512:(s + 1) * 512, :]
                        .rearrange("(j p) t -> p j t", p=128),
                        in_=qn4)

                # ---- a/b rows -> beta, g, G ----
                ps_ab = pb_.tile([128, TB], F32, tag="ab", name="ps_ab")
                for k in range(KS):
                    nc.tensor.matmul(ps_ab, w_ab_sb[:, k, :], x_t[:, k, :],
                                     start=(k == 0), stop=(k == KS - 1))
                zr = cp.tile([1, 128], F32, tag="zr", name="zr")
                nc.vector.memset(zr, 0.0)
                for s in range(2):
                    R = 32 * s
                    bt_ = cp.tile([1, TB], F32, tag="abtmp", name="bt_")
                    nc.scalar.activation(out=bt_, in_=ps_ab[64 + R:65 + R, :],
                                         func=AF.Exp, scale=-1.0)
                    nc.scalar.activation(out=bt_, in_=bt_, func=AF.Ln, bias=1.0)
                    nc.scalar.activation(out=brows[R:R + 1, t0:t0 + TB], in_=bt_,
                                         func=AF.Exp, scale=-1.0)
                    nc.vector.tensor_scalar_mul(brows_neg[R:R + 1, t0:t0 + TB],
                                                brows[R:R + 1, t0:t0 + TB], -1.0)
                    gtmp = cp.tile([1, TB], F32, tag="abtmp", name="gtmp")
                    nc.scalar.activation(out=gtmp, in_=ps_ab[R:R + 1, :],
                                         func=AF.Exp, bias=cst_sb[R:R + 1, 56:57])
                    nc.scalar.activation(out=gtmp, in_=gtmp, func=AF.Ln, bias=1.0)
                    nc.vector.tensor_scalar_mul(gtmp, gtmp, cst_sb[R:R + 1, 57:58])
                    for cc in range(TB // C):
                        a0 = t0 + cc * C
                        nc.vector.tensor_tensor_scan(
                            out=Grows[R:R + 1, a0:a0 + C],
                            data0=gtmp[:, cc * C:(cc + 1) * C],
                            data1=zr[:, 0:C], initial=0.0, op0=ALU.add, op1=ALU.add)

        # ========= PHASE B + C pools (entered early so A2 overlaps) =========
        with ExitStack() as bctx:
            bp = bctx.enter_context(tc.tile_pool(name="bp", bufs=4))
            ep = bctx.enter_context(tc.tile_pool(name="ep", bufs=4))
            vp = bctx.enter_context(tc.tile_pool(name="vp", bufs=3))
            pm = bctx.enter_context(tc.tile_pool(name="pm", bufs=2, space="PSUM"))
            pt = bctx.enter_context(tc.tile_pool(name="pt", bufs=2, space="PSUM"))
            pc = bctx.enter_context(tc.tile_pool(name="pc", bufs=2, space="PSUM"))

            # ---- pass A2a: v projections + conv + silu ----
            with ExitStack() as actx:
                wpv = actx.enter_context(tc.tile_pool(name="wpv", bufs=1))
                xpv = actx.enter_context(tc.tile_pool(name="xpv", bufs=2))
                cpv = actx.enter_context(tc.tile_pool(name="cpv", bufs=2))
                pav = actx.enter_context(tc.tile_pool(name="pav", bufs=2, space="PSUM"))
                conv_silu2 = make_conv_silu(cpv)

                w_v_sb = wpv.tile([128, KS, 768], BF16)
                nc.sync.dma_start(out=w_v_sb, in_=w_v[:].rearrange("(k p) c -> p k c", p=128))

                for tt in range(NT):
                    t0 = tt * TB
                    x_t = xpv.tile([128, KS, TB], BF16, name="x_t2")
                    nc.sync.dma_start(out=x_t, in_=xt[:, t0:t0 + TB].rearrange("(k p) t -> p k t", p=128))
                    for u in range(3):
                        vsl = cpv.tile([128, 2, TB], BF16, tag="vsl", name="vsl")
                        for ft in range(2):
                            ps = pav.tile([128, TB], F32, tag="proj", name="ps_v")
                            col = u * 256 + ft * 128
                            for k in range(KS):
                                nc.tensor.matmul(ps, w_v_sb[:, k, col:col + 128],
                                                 x_t[:, k, :], start=(k == 0), stop=(k == KS - 1))
                            conv_silu2(ps, ("v", u, ft), 32 + u * 8 + ft * 4,
                                       out=vsl[:, ft, :])
                        nc.sync.dma_start(
                            out=v_drs[tt][u * 256:(u + 1) * 256, :]
                            .rearrange("(f p) t -> p f t", p=128),
                            in_=vsl)

            # ---- pass A2b: gate projections + silu ----
            with ExitStack() as actx:
                wpg = actx.enter_context(tc.tile_pool(name="wpg", bufs=1))
                xpg = actx.enter_context(tc.tile_pool(name="xpg", bufs=2))
                cpg = actx.enter_context(tc.tile_pool(name="cpg", bufs=2))
                pag = actx.enter_context(tc.tile_pool(name="pag", bufs=2, space="PSUM"))

                w_g_sb = wpg.tile([128, KS, 768], BF16)
                nc.sync.dma_start(out=w_g_sb, in_=w_g[:].rearrange("(k p) c -> p k c", p=128))

                for tt in range(NT):
                    t0 = tt * TB
                    x_t = xpg.tile([128, KS, TB], BF16, name="x_t3")
                    nc.sync.dma_start(out=x_t, in_=xt[:, t0:t0 + TB].rearrange("(k p) t -> p k t", p=128))
                    for st in range(TB // 128):
                        gl768 = cpg.tile([128, 768], BF16, tag="gl768", name="gl768")
                        for u in range(3):
                            ps = pag.tile([128, 256], F32, tag="gate", name="ps_g")
                            col = u * 256
                            for k in range(KS):
                                nc.tensor.matmul(ps, x_t[:, k, st * 128:(st + 1) * 128],
                                                 w_g_sb[:, k, col:col + 256],
                                                 start=(k == 0), stop=(k == KS - 1))
                            ge = cpg.tile([128, 256], F32, tag="ge", name="ge")
                            nc.scalar.activation(out=ge, in_=ps, func=AF.Exp, scale=-1.0)
                            nc.scalar.activation(out=ge, in_=ge, func=AF.Ln, bias=1.0)
                            ge2 = cpg.tile([128, 256], BF16, tag="ge2", name="ge2")
                            nc.scalar.activation(out=ge2, in_=ge, func=AF.Exp, scale=-1.0)
                            nc.vector.tensor_mul(gl768[:, u * 256:(u + 1) * 256], ps, ge2)
                        nc.sync.dma_start(
                            out=gate_drs[tt][st * 128:(st + 1) * 128, :],
                            in_=gl768)

            # ================= PHASE B + C =================
            for c in range(NCH):
                a0 = c * C
                sl = slice(a0, a0 + C)
                # shared per-chunk: transpose [G;beta] -> cols
                gb = ep.tile([97, C], F32, tag="gb")
                nc.gpsimd.memset(gb, 0.0)
                nc.gpsimd.tensor_copy(out=gb[0:1, :], in_=Grows[0:1, sl])
                nc.gpsimd.tensor_copy(out=gb[32:33, :], in_=Grows[32:33, sl])
                nc.gpsimd.tensor_copy(out=gb[64:65, :], in_=brows[0:1, sl])
                nc.gpsimd.tensor_copy(out=gb[96:97, :], in_=brows[32:33, sl])
                ps_t = pm.tile([128, 97], F32, tag="pm")
                nc.tensor.transpose(ps_t, gb, idf_sb[0:97, 0:97])
                cols = ep.tile([128, 97], F32, tag="cols")
                nc.any.tensor_copy(out=cols, in_=ps_t)

                slot = {}
                for s in range(2):
                    # q/k chunk slices (one DMA: j = qk*2 + ft)
                    qks = vp.tile([128, 4, C], BF16, tag="qks")
                    cc4 = (c % 4) * C
                    nc.sync.dma_start(out=qks,
                                      in_=qk_drs[c // 4][s * 512:(s + 1) * 512,
                                                         cc4:cc4 + C]
                                      .rearrange("(j p) t -> p j t", p=128))

                    gcol = cols[:, 32 * s:32 * s + 1]
                    bcol = cols[:, 64 + 32 * s:65 + 32 * s]
                    # G_C broadcast col + derived scalars
                    ps_gc = pm.tile([128, 1], F32, tag="pm")
                    R = 32 * s
                    nc.tensor.matmul(ps_gc, ones_f_row[R:R + 1, :],
                                     Grows[R:R + 1, a0 + C - 1:a0 + C],
                                     start=True, stop=True)
                    gcc = ep.tile([128, 1], F32, tag="gcc")
                    nc.any.tensor_copy(out=gcc, in_=ps_gc)
                    eGC = ep.tile([128, 1], F32, tag="eGC")
                    nc.scalar.activation(out=eGC, in_=gcc, func=AF.Exp)
                    ktsc = ep.tile([128, 1], F32, tag="ktsc")
                    nc.scalar.activation(out=ktsc, in_=gcol, func=AF.Exp,
                                         scale=-1.0, bias=gcc)
                    nc.vector.tensor_mul(ktsc, ktsc, bcol)
                    egcol = ep.tile([128, 1], F32, tag="egcol")
                    nc.scalar.activation(out=egcol, in_=gcol, func=AF.Exp)

                    # replicated rows
                    ps_gr = pm.tile([128, C], F32, tag="pm")
                    nc.tensor.matmul(ps_gr, ones_f_row[R:R + 1, :], Grows[R:R + 1, sl],
                                     start=True, stop=True)

                    # E matrices (fused): strict-lower for Mn, incl-upper for
                    # MnT (diag zeroed via mask) and PT
                    ps_brn = pm.tile([128, C], F32, tag="pm")
                    nc.tensor.matmul(ps_brn, ones_f_row[R:R + 1, :],
                                     brows_neg[R:R + 1, sl], start=True, stop=True)
                    dd = ep.tile([128, C], F32, tag="dd")
                    nc.vector.scalar_tensor_tensor(out=dd, in0=ps_gr, scalar=gcol,
                                                   in1=msk_sb[:, 0:128],
                                                   op0=ALU.subtract, op1=ALU.add)
                    es = ep.tile([128, C], F32, tag="es")
                    nc.scalar.activation(out=es, in_=dd, func=AF.Exp, scale=-1.0)
                    dd3 = ep.tile([128, C], F32, tag="dd3")
                    nc.vector.scalar_tensor_tensor(out=dd3, in0=ps_gr, scalar=gcol,
                                                   in1=msk_sb[:, 128:256],
                                                   op0=ALU.subtract, op1=ALU.add)
                    eupi = ep.tile([128, C], F32, tag="eupi")
                    nc.scalar.activation(out=eupi, in_=dd3, func=AF.Exp)

                    esb = ep.tile([128, C], F32, tag="esb")
                    nc.vector.tensor_mul(esb, es, ps_brn)       # -beta_j e^{Gi-Gj} strict-low
                    eupn = ep.tile([128, C], F32, tag="eupn")   # -beta_i e^{Gj-Gi} strict-up
                    nc.vector.scalar_tensor_tensor(out=eupn, in0=eupi, scalar=bcol,
                                                   in1=msk_sb[:, 256:384],
                                                   op0=ALU.mult, op1=ALU.mult)
                    nc.vector.tensor_scalar_mul(eupi, eupi, bcol)

                    # KK / KQ
                    ps_kk = pm.tile([128, C], F32, tag="pm")
                    for ft in range(2):
                        nc.tensor.matmul(ps_kk, qks[:, 2 + ft, :], qks[:, 2 + ft, :],
                                         start=(ft == 0), stop=(ft == 1))
                    ps_kq = pm.tile([128, C], F32, tag="pm")
                    for ft in range(2):
                        nc.tensor.matmul(ps_kq, qks[:, 2 + ft, :], qks[:, ft, :],
                                         start=(ft == 0), stop=(ft == 1))
                    Mn = bp.tile([128, C], BF16, tag="Mn")
                    nc.vector.tensor_mul(Mn, esb, ps_kk)
                    MnT = bp.tile([128, C], BF16, tag="MnT")
                    nc.vector.tensor_mul(MnT, eupn, ps_kk)
                    PT = bp.tile([128, C], BF16, tag="PT")
                    nc.vector.tensor_mul(PT, eupi, ps_kq)

                    # T-chain: A = Tmat^T
                    A = bp.tile([128, C], BF16, tag="A0")
                    nc.vector.tensor_add(A, idb_sb, MnT)
                    ps_x = pm.tile([128, C], F32, tag="pm")
                    nc.tensor.matmul(ps_x, MnT, Mn, start=True, stop=True)
                    X = bp.tile([128, C], BF16, tag="X")
                    nc.any.tensor_copy(out=X, in_=ps_x)
                    ps_a = pm.tile([128, C], F32, tag="pm")
                    nc.tensor.matmul(ps_a, X, A, start=True, stop=True)
                    A2 = bp.tile([128, C], BF16, tag="A1")
                    nc.vector.tensor_add(A2, ps_a, A)
                    A = A2

                    # K time-major via PE transpose
                    ktm = bp.tile([128, 256], BF16, tag="ktm")
                    for ft in range(2):
                        ps_tr = pt.tile([128, 128], BF16, tag="tr")
                        nc.tensor.transpose(ps_tr, qks[:, 2 + ft, :], idb_sb)
                        nc.any.tensor_copy(out=ktm[:, ft * 128:(ft + 1) * 128], in_=ps_tr)
                    kgn = bp.tile([128, 256], BF16, tag="kgn")
                    nc.gpsimd.tensor_scalar(out=kgn, in0=ktm, scalar1=egcol,
                                            scalar2=-1.0, op0=ALU.mult, op1=ALU.mult)
                    ktl = bp.tile([128, 256], BF16, tag="ktl")
                    nc.gpsimd.tensor_scalar_mul(ktl, ktm, ktsc)
                    tkgn = bp.tile([128, 256], BF16, tag="tkgn")
                    for ft in range(2):
                        ps_k = pm.tile([128, C], F32, tag="pm")
                        nc.tensor.matmul(ps_k, kgn[:, ft * 128:(ft + 1) * 128], A,
                                         start=True, stop=True)
                        nc.any.tensor_copy(out=tkgn[:, ft * 128:(ft + 1) * 128], in_=ps_k)
                    egr = ep.tile([1, C], BF16, tag="egr")
                    nc.scalar.activation(out=egr, in_=Grows[R:R + 1, sl], func=AF.Exp)
                    ps_eg = pm.tile([128, C], F32, tag="pm")
                    nc.tensor.matmul(ps_eg, ones_b_row[0:1, :], egr,
                                     start=True, stop=True)
                    qgt = bp.tile([128, 256], BF16, tag="qgt")
                    for ft in range(2):
                        nc.vector.tensor_mul(qgt[:, ft * 128:(ft + 1) * 128],
                                             qks[:, ft, :], ps_eg)
                    slot[s] = dict(A=A, PT=PT, tkgn=tkgn, qgt=qgt, ktl=ktl, eGC=eGC)

                # ---- per-unit chain ----
                obfs = []
                for u in range(3):
                    s = SU[u]
                    sd = slot[s]
                    vfa = vp.tile([128, 2, C], BF16, tag="vfa")
                    cc4 = (c % 4) * C
                    nc.sync.dma_start(out=vfa,
                                      in_=v_drs[c // 4][u * 256:(u + 1) * 256,
                                                        cc4:cc4 + C]
                                      .rearrange("(f p) t -> p f t", p=128))
                    vtm = vp.tile([128, 256], BF16, tag="vtm")
                    for ft in range(2):
                        ps_tr = pt.tile([128, 128], BF16, tag="tr")
                        nc.tensor.transpose(ps_tr, vfa[:, ft, :], idb_sb)
                        nc.any.tensor_copy(out=vtm[:, ft * 128:(ft + 1) * 128], in_=ps_tr)

                    ps_u = pc.tile([128, DVU], F32, tag="pc")
                    nc.tensor.matmul(ps_u, sd["A"], vtm, start=True, stop=(c == 0))
                    if c > 0:
                        for ft in range(2):
                            nc.tensor.matmul(ps_u, sd["tkgn"][:, ft * 128:(ft + 1) * 128],
                                             S_b[u][:, ft, :], start=False, stop=(ft == 1))
                    u_sb = vp.tile([128, DVU], BF16, tag="usb")
                    nc.vector.tensor_copy(out=u_sb, in_=ps_u)

                    ps_o = pc.tile([128, DVU], F32, tag="pc")
                    if c > 0:
                        for ft in range(2):
                            nc.tensor.matmul(ps_o, sd["qgt"][:, ft * 128:(ft + 1) * 128],
                                             S_b[u][:, ft, :], start=(ft == 0), stop=False)
                    nc.tensor.matmul(ps_o, sd["PT"], u_sb, start=(c == 0), stop=True)
                    obf = vp.tile([128, DVU], BF16, tag="obf", name=f"obf{u}")
                    nc.vector.tensor_copy(out=obf, in_=ps_o)
                    if u == 2:
                        nc.sync.dma_start(out=o_drs[c][:], in_=obf)
                    obfs.append(obf)
                    trash = vp.tile([128, DVU], BF16, tag="trash")
                    nc.scalar.activation(out=trash, in_=ps_o, func=AF.Square,
                                         accum_out=ssq[:, u * NCH + c:u * NCH + c + 1])

                    for ft in range(2):
                        ps_s = pc.tile([128, DVU], F32, tag="pc")
                        nc.tensor.matmul(ps_s, sd["ktl"][:, ft * 128:(ft + 1) * 128],
                                         u_sb, start=True, stop=True)
                        nc.vector.scalar_tensor_tensor(
                            out=S_f[u][:, ft, :], in0=S_f[u][:, ft, :],
                            scalar=sd["eGC"], in1=ps_s, op0=ALU.mult, op1=ALU.add)
                    nc.gpsimd.tensor_copy(out=S_b[u], in_=S_f[u])

                # ---- inline RMS norm + gate for the full head (units 0,1) ----
                gt01 = vp.tile([128, 512], BF16, tag="gt01")
                nc.sync.dma_start(out=gt01,
                                  in_=gate_drs[c // 4][(c % 4) * C:(c % 4 + 1) * C, 0:512])
                totA = ep.tile([128, 1], F32, tag="totA")
                nc.vector.tensor_add(totA, ssq[:, c:c + 1], ssq[:, NCH + c:NCH + c + 1])
                rsqA = ep.tile([128, 1], F32, tag="rsqA")
                nc.scalar.activation(out=rsqA, in_=totA, func=AF.Ln,
                                     scale=float(1.0 / DV), bias=cst_sb[:, 61:62])
                nc.scalar.activation(out=rsqA, in_=rsqA, func=AF.Exp, scale=-0.5)
                on01 = vp.tile([128, 512], BF16, tag="on01")
                for u in range(2):
                    nc.vector.scalar_tensor_tensor(
                        out=on01[:, u * 256:(u + 1) * 256], in0=obfs[u], scalar=rsqA,
                        in1=gt01[:, u * 256:(u + 1) * 256], op0=ALU.mult, op1=ALU.mult)
                nc.sync.dma_start(out=o2_drs[c][:, 0:512], in_=on01)

                if c == NCH // 2 - 1:
                    nc.sync.dma_start(out=ssq_cc_in[0][:],
                                      in_=ssq[:, 2 * NCH:2 * NCH + NCH // 2])
                    nc.gpsimd.collective_compute(
                        "AllReduce", mybir.AluOpType.add,
                        replica_groups=[[0, 1], [2, 3], [4, 5], [6, 7]],
                        ins=[ssq_cc_in[0][:]], outs=[ssq_cc_out[0][:]])
                    nc.sync.dma_start(out=ssq2x[:, 0:NCH // 2], in_=ssq_cc_out[0][:])

        # ---- ssq pair exchange (2nd half) + unit-2 norm pass ----
        nc.sync.dma_start(out=ssq_cc_in[1][:],
                          in_=ssq[:, 2 * NCH + NCH // 2:3 * NCH])
        nc.gpsimd.collective_compute(
            "AllReduce", mybir.AluOpType.add,
            replica_groups=[[0, 1], [2, 3], [4, 5], [6, 7]],
            ins=[ssq_cc_in[1][:]], outs=[ssq_cc_out[1][:]])
        nc.sync.dma_start(out=ssq2x[:, NCH // 2:NCH], in_=ssq_cc_out[1][:])

        with ExitStack() as cctx:
            np_ = cctx.enter_context(tc.tile_pool(name="np", bufs=3))
            for c in range(NCH):
                a0 = c * C
                sl = slice(a0, a0 + C)
                rsqB = np_.tile([128, 1], F32, tag="rsqB")
                nc.scalar.activation(out=rsqB, in_=ssq2x[:, c:c + 1], func=AF.Ln,
                                     scale=float(1.0 / DV), bias=cst_sb[:, 61:62])
                nc.scalar.activation(out=rsqB, in_=rsqB, func=AF.Exp, scale=-0.5)
                ot2 = np_.tile([128, 256], BF16, tag="ot2")
                nc.sync.dma_start(out=ot2, in_=o_drs[c][:])
                gt2 = np_.tile([128, 256], BF16, tag="gt2")
                nc.sync.dma_start(out=gt2,
                                  in_=gate_drs[c // 4][(c % 4) * C:(c % 4 + 1) * C, 512:768])
                on2 = np_.tile([128, 256], BF16, tag="on2")
                nc.vector.scalar_tensor_tensor(out=on2, in0=ot2, scalar=rsqB,
                                               in1=gt2, op0=ALU.mult, op1=ALU.mult)
                nc.sync.dma_start(out=o2_drs[c][:, 512:768], in_=on2)

        # ================= PHASE D =================
        with ExitStack() as dctx:
            dp = dctx.enter_context(tc.tile_pool(name="dp", bufs=3))
            wop = dctx.enter_context(tc.tile_pool(name="wop", bufs=1))
            pd = dctx.enter_context(tc.tile_pool(name="pd", bufs=3, space="PSUM"))
            w_o_sb = wop.tile([128, 6, HID], BF16)
            nc.sync.dma_start(out=w_o_sb, in_=w_o[:].rearrange("(r p) n -> p r n", p=128))
            for st4 in range(8):
                for ti in range(4):
                    rows = st4 * 512 + ti * 128
                    o2l = dp.tile([128, 768], BF16, tag="o2l")
                    nc.sync.dma_start(out=o2l, in_=o2_drs[rows // 128][:])
                    otl = dp.tile([128, 6, 128], BF16, tag="otl")
                    for r in range(6):
                        ps_tr = pd.tile([128, 128], BF16, tag="tr")
                        nc.tensor.transpose(ps_tr, o2l[:, r * 128:(r + 1) * 128], idb_sb)
                        if r % 2 == 0:
                            nc.scalar.copy(out=otl[:, r, :], in_=ps_tr)
                        else:
                            nc.vector.tensor_copy(out=otl[:, r, :], in_=ps_tr)
                    pob = dp.tile([128, 2048], BF16, tag="pob")
                    for nt_ in range(4):
                        ps = pd.tile([128, 512], F32, tag="pd")
                        for r in range(6):
                            nc.tensor.matmul(ps, otl[:, r, :],
                                             w_o_sb[:, r, nt_ * 512:(nt_ + 1) * 512],
                                             start=(r == 0), stop=(r == 5))
                        eng = nc.scalar if nt_ % 2 == 0 else nc.vector
                        if nt_ % 2 == 0:
                            nc.scalar.copy(out=pob[:, nt_ * 512:(nt_ + 1) * 512], in_=ps)
                        else:
                            nc.vector.tensor_copy(out=pob[:, nt_ * 512:(nt_ + 1) * 512], in_=ps)
                    nc.sync.dma_start(out=po_st[st4][ti * 128:(ti + 1) * 128, :],
                                      in_=pob)
                nc.gpsimd.collective_compute(
                    "ReduceScatter", mybir.AluOpType.add,
                    replica_groups=[[0, 1, 2, 3], [4, 5, 6, 7]],
                    ins=[po_st[st4][:]], outs=[rs_st[st4][:]])
                rsb = dp.tile([128, HID], BF16, tag="rsb")
                nc.sync.dma_start(out=rsb, in_=rs_st[st4][:])
                rsf = dp.tile([128, HID], F32, tag="rsf")
                nc.vector.tensor_copy(out=rsf, in_=rsb)
                nc.sync.dma_start(out=out[st4 * 128:(st4 + 1) * 128, :], in_=rsf)

    import concourse.bacc as bacc_mod
    _orig_gat = bacc_mod.get_activation_tables
    def _pin_tables(arch):
        t = _orig_gat(arch)
        return {k: (v if k == "natural_log_exp_and_others" else set())
                for k, v in t.items()}
    bacc_mod.get_activation_tables = _pin_tables
    try:
        nc.compile()
    finally:
        bacc_mod.get_activation_tables = _orig_gat
    _BUILD_CACHE["nc"] = nc
    return nc


def _prep_core_inputs(ins, core):
    """Pack per-core input arrays. ins: dict of fp32 numpy arrays."""
    b, g = core // 4, core % 4
    hA, hB, dvhB = GROUPS[g]
    units = [(hA, 0), (hA, 1), (hB, dvhB)]
    heads = [hA, hB]

    x = np.asarray(ins["hidden_states"], np.float32)[b]        # [T, HID]
    xt = np.ascontiguousarray(x.T).astype(BF)                  # [HID, T]

    w_qk = np.zeros((HID, 1024), np.float32)
    w_ab = np.zeros((HID, 128), np.float32)
    for s, h in enumerate(heads):
        w_qk[:, s * 512:s * 512 + 256] = ins["Wq"][h * DK:(h + 1) * DK].T
        w_qk[:, s * 512 + 256:s * 512 + 512] = ins["Wk"][h * DK:(h + 1) * DK].T
        w_ab[:, 32 * s] = ins["Wa"][h]
        w_ab[:, 64 + 32 * s] = ins["Wb"][h]

    w_v = np.zeros((HID, 768), np.float32)
    w_g = np.zeros((HID, 768), np.float32)
    w_o = np.zeros((768, HID), np.float32)
    for u, (h, dvh) in enumerate(units):
        r = slice(h * DV + dvh * 256, h * DV + dvh * 256 + 256)
        w_v[:, u * 256:(u + 1) * 256] = ins["Wv"][r].T
        w_g[:, u * 256:(u + 1) * 256] = ins["Wg"][r].T
        nw = ins["norm_w"][dvh * 256:(dvh + 1) * 256]
        w_o[u * 256:(u + 1) * 256, :] = (ins["Wo"][:, r].T * nw[:, None])

    cst = np.zeros((128, 64), np.float32)
    for s, h in enumerate(heads):
        for qk, cw in ((0, ins["conv_wq"]), (1, ins["conv_wk"])):
            for ft in range(2):
                for j in range(4):
                    cst[:, s * 16 + qk * 8 + ft * 4 + j] = \
                        cw[h * DK + ft * 128:h * DK + (ft + 1) * 128, j]
    for u, (h, dvh) in enumerate(units):
        for ft in range(2):
            for j in range(4):
                cst[:, 32 + u * 8 + ft * 4 + j] = \
                    ins["conv_wv"][h * DV + dvh * 256 + ft * 128:
                                   h * DV + dvh * 256 + (ft + 1) * 128, j]
    for s, h in enumerate(heads):
        cst[32 * s, 56] = ins["dt_bias"][h]
        cst[32 * s, 57] = -np.exp(ins["A_log"][h])
    cst[:, 58] = 1.0
    cst[:, 59] = DK * L2_EPS
    cst[:, 60] = L2_EPS
    cst[:, 61] = NORM_EPS

    ii, jj = np.mgrid[0:128, 0:128]
    msk = np.zeros((128, 384), np.float32)
    msk[:, 0:128] = np.where(jj >= ii, 1e9, 0.0)       # pre-exp(-x) strict-lower
    msk[:, 128:256] = np.where(jj < ii, -1e9, 0.0)     # pre-exp(+x) incl-upper
    msk[:, 256:384] = np.where(jj > ii, -1.0, 0.0)     # -(strict upper 0/1)

    idf = np.eye(128, dtype=np.float32)

    return {
        "xt": np.ascontiguousarray(xt),
        "w_qk": np.ascontiguousarray(w_qk.astype(BF)),
        "w_ab": np.ascontiguousarray(w_ab.astype(BF)),
        "w_v": np.ascontiguousarray(w_v.astype(BF)),
        "w_g": np.ascontiguousarray(w_g.astype(BF)),
        "w_o": np.ascontiguousarray(w_o.astype(BF)),
        "cst": cst,
        "msk": msk,
        "idf": idf,
        "idb": np.ascontiguousarray(idf.astype(BF)),
    }


LAST_RESULTS = None


def _run_device(ins, trace=False):
    global LAST_RESULTS
    from concourse.bass_utils import run_bass_kernel_spmd
    nc = _build()
    in_maps = [_prep_core_inputs(ins, c) for c in range(N_CORES)]
    res = run_bass_kernel_spmd(nc, in_maps, list(range(N_CORES)), trace=trace)
    LAST_RESULTS = res
    B = 2
    full = np.empty((B, T, HID), np.float32)
    for core in range(N_CORES):
        b, g = core // 4, core % 4
        o = res.results[core]["out"]                # [1024, HID]
        for st4 in range(8):
            full[b, st4 * 512 + g * 128: st4 * 512 + (g + 1) * 128] = \
                o[st4 * 128:(st4 + 1) * 128]
    return full


def kernel(**inputs):
    ins = {k: np.asarray(v, np.float32) for k, v in inputs.items()}
    return _run_device(ins)
